# revision 17
# baseline (speedup 1.0000x reference)
"""BuildingBlockEmbedder GNN kernel for trn2 — shared library.

Layout: feature-on-partition ("transposed") everywhere on device.
Per core: 64 building blocks x 32 atoms = 2048 nodes, 40960 edges.
"""
import numpy as np
import ml_dtypes

BF16 = ml_dtypes.bfloat16

# problem constants
NUM_GAUSS = 64
MAX_R = 5.0
L = 4
C = 256            # node/hidden channels
NPB = 32           # atoms per block
K = 20             # neighbors
B = 512            # blocks
N = B * NPB
E = N * K
GAUSS_COEFF = -0.5 / (MAX_R / (NUM_GAUSS - 1)) ** 2
NCORES = 8
BPC = B // NCORES          # 64 blocks per core
NPC = BPC * NPB            # 2048 nodes per core
EPB = NPB * K              # 640 edges per block
HALF = EPB // 2            # 320-edge matmul unit

# ---------------------------------------------------------------- host prep

def host_prep(local_coords, atom_types, edge_index, batch_bb, atom_embed, offset,
              We1, be1, We2, be2, Wn1, bn1, Wn2, bn2):
    """Build per-core device inputs from full problem inputs (all numpy)."""
    pos = np.asarray(local_coords, np.float32)
    types = np.asarray(atom_types).astype(np.int64)
    ei = np.asarray(edge_index).astype(np.int64)
    We1 = np.asarray(We1, np.float32); be1 = np.asarray(be1, np.float32)
    We2 = np.asarray(We2, np.float32); be2 = np.asarray(be2, np.float32)
    Wn1 = np.asarray(Wn1, np.float32); bn1 = np.asarray(bn1, np.float32)
    Wn2 = np.asarray(Wn2, np.float32); bn2 = np.asarray(bn2, np.float32)
    emb = np.asarray(atom_embed, np.float32)

    row, col = ei[0], ei[1]
    # structural assumptions from the reference graph builder
    assert np.array_equal(row, np.repeat(np.arange(N, dtype=np.int64), K)), \
        "edge rows must be repeat(arange(N), K)"
    assert np.all(col // NPB == row // NPB), "edges must stay within blocks"

    dvec = pos[col] - pos[row]
    d = np.sqrt((dvec * dvec).sum(-1))          # [E] Angstrom
    assert d.max() < MAX_R - 0.55, f"d.max()={d.max()}: last gaussian not negligible"
    radial = (0.01 * d * d).astype(np.float32)  # ANG_TO_NM^2 * d^2
    # gaussians 0..62 (63rd is exp(<-30) ~= 0 for all d here; its row carries radial)
    off = np.asarray(offset, np.float32)
    gauss = np.exp(GAUSS_COEFF * (d[:, None] - off[None, :63]) ** 2).astype(np.float32)

    # CRG [B, 128, EPB]: rows 0-31 C_sel, 32-63 R_sel, 64 radial, 65-127 gauss
    col_local = (col - (row // NPB) * NPB).astype(np.int32).reshape(B, EPB)
    crg = np.zeros((B, 128, EPB), np.float32)
    e_ar = np.arange(EPB)
    r_sel = np.zeros((NPB, EPB), np.float32)
    r_sel[e_ar // K, e_ar] = 1.0
    for b in range(B):
        crg[b, col_local[b], e_ar] = 1.0      # C_sel
    crg[:, 32:64, :] = r_sel[None]
    crg[:, 64, :] = radial.reshape(B, EPB)
    crg[:, 65:, :] = gauss.reshape(B, EPB, 63).transpose(0, 2, 1)
    crg = crg.astype(BF16)

    h0 = emb[types - 1]                        # [N, C] f32
    h0T = h0.reshape(NCORES, NPC, C).transpose(0, 2, 1).reshape(
        NCORES, 2, 128, NPC).copy()            # [core, chunk, 128, 2048]

    def chunks_lhsT(w):   # w [L, 256, 256] -> [L, kc, mc, 128, 128] bf16
        return np.ascontiguousarray(
            w.reshape(L, 2, 128, 2, 128).transpose(0, 1, 3, 2, 4)).astype(BF16)

    w1b = np.ascontiguousarray(
        We1[:, 256:512, :].reshape(L, 2, 128, 256))      # rhs [L, kc, 128, 256]
    w1a = np.ascontiguousarray(We1[:, 0:256, :].reshape(L, 2, 128, 256))
    w1ab = np.stack([w1b, w1a], axis=1).astype(BF16)     # [L, 2(b,a), kc, 128, 256]
    w1dp = np.concatenate([We1[:, 512:513, :], We1[:, 513:576, :]],
                          axis=1).astype(BF16)           # [L, 64, 256]
    we2 = chunks_lhsT(We2)
    wn1a = chunks_lhsT(Wn1[:, 0:256, :])
    wn1b = chunks_lhsT(Wn1[:, 256:512, :])
    wn2 = chunks_lhsT(Wn2)
    # bias K=1 lhsT rows [L, 2(bn1,bn2), mc, 1, 128] bf16
    bnrow = np.stack([bn1, bn2], axis=1).reshape(L, 2, 2, 1, 128).astype(BF16)
    # ACT/DVE bias columns [128, 4*L*2]; col = (j*L + l)*2 + mc
    # j: 0 = be1, 1 = be2, 2 = -be2, 3 = -be1
    be12 = np.zeros((128, 4 * L * 2), np.float32)
    for j, bb in enumerate([be1, be2, -be2, -be1]):
        for l in range(L):
            for mc in range(2):
                be12[:, (j * L + l) * 2 + mc] = bb[l, mc * 128:(mc + 1) * 128]

    def pmaj(w):  # [L, kc, mc, 128p, 128q] -> [128, L*kc*mc*128]
        return np.ascontiguousarray(w.transpose(3, 0, 1, 2, 4).reshape(128, -1))

    # v2 extras: We2 as rhs [128, L*2*256]; be2 repeated row; Ssel matrices
    we2r = np.ascontiguousarray(
        We2.reshape(L, 2, 128, 256).transpose(2, 0, 1, 3).reshape(128, -1)).astype(BF16)
    be2row = np.ascontiguousarray(
        np.repeat(be2[:, None, :], 2, axis=1).reshape(1, L * 512)).astype(BF16)
    ssel = np.zeros((128, 5 * NPB), np.float32)
    for ec in range(5):
        j = np.arange(128)
        ssel[j, ec * NPB + (ec * 128 + j) // K] = 1.0
    ssel = np.ascontiguousarray(ssel).astype(BF16)

    shared = dict(
        we2r=we2r, be2row=be2row, ssel=ssel,
        be2_nonzero=np.asarray([1.0 if np.any(be2 != 0) else 0.0], np.float32),
        we2=pmaj(we2),
        w1ab=np.ascontiguousarray(w1ab.transpose(3, 0, 1, 2, 4).reshape(128, -1)),
        w1dp=np.ascontiguousarray(w1dp.transpose(1, 0, 2).reshape(64, -1)),
        wn1a=pmaj(wn1a), wn1b=pmaj(wn1b), wn2=pmaj(wn2),
        bnrow=np.ascontiguousarray(bnrow.transpose(3, 0, 1, 2, 4).reshape(1, -1)),
        be12=be12)
    per_core = []
    for c in range(NCORES):
        m = dict(h0T=h0T[c], crg=crg[c * BPC:(c + 1) * BPC])
        m.update(shared)
        per_core.append(m)
    return per_core


def host_unshard(results):
    """results: list of 8 dicts with 'poolT' [2,128,nb] -> full [B, 256] f32."""
    outs = []
    for r in results:
        pt = np.asarray(r["poolT"], np.float32)      # [2, 128, nb]
        nb = pt.shape[2]
        outs.append(pt.reshape(256, nb).T)           # [nb, 256]
    return np.concatenate(outs, axis=0)


# ------------------------------------------------------------ tile drain fix

def apply_tilefix():
    """This container's walrus allows only ONE sem-wait on an SP Drain —
    split the Tile tail-drain waits across serial drains."""
    import concourse.mybir as mybir
    import concourse.tile as tile
    from concourse.tile import ScopedClock

    if getattr(tile.TileContext, "_drain_fix_applied", False):
        return

    def _split(self, tick_clock, wait_clock):
        d = self.nc.sync.drain()
        wait_clock.add_sem_waits(d.ins, ScopedClock({None: tick_clock.global_clock}))
        ws = list(d.ins.sync_info.on_wait) if d.ins.sync_info is not None else []
        if len(ws) > 1:
            d.ins.sync_info.on_wait = ws[:1]
            for w in ws[1:]:
                e = self.nc.sync.drain()
                e.ins.sync_info = mybir.SyncInfo(on_update=[], on_wait=[w])
        self.nc.all_engine_barrier()
        assert self.sems is not None
        popped = self.nc._tile_sem_poison_stack.pop()
        assert popped is self._sem_poison
        self.nc.clear_and_free_semaphores(list(self.sems.allocated().values()))
        self.nc.all_engine_barrier()

    tile.TileContext._drain_and_barrier = _split
    tile.TileContext._drain_fix_applied = True


# ---------------------------------------------------- wait-splitting post-pass

def split_waits(nc, cap=1, cap_sp=1):
    """walrus in this container caps sem-waits per instruction. Hoist excess
    waits onto same-engine NOPs emitted just before the instruction."""
    import concourse.mybir as mybir
    k = 0
    for fn in nc.m.functions:
        for bb in fn.blocks:
            out = []
            for inst in bb.instructions:
                si = inst.sync_info
                ws = list(si.on_wait) if si is not None else []
                c = cap_sp if inst.engine == mybir.EngineType.SP else cap
                if len(ws) > c:
                    keep = ws[:c] if c > 0 else []
                    rest = ws[c:] if c > 0 else ws
                    while rest:
                        chunk, rest = rest[:max(c, 1)], rest[max(c, 1):]
                        nop = mybir.InstNoOp(
                            name=f"wsplit-{k}", engine=inst.engine,
                            sync_info=mybir.SyncInfo(on_wait=chunk, on_update=[]),
                            bass_nofuse=True)
                        k += 1
                        out.append(nop)
                    inst.sync_info.on_wait = keep
                out.append(inst)
            bb.instructions[:] = out
    return k


# ------------------------------------------------------------- bass builder

def build_nc(nb=BPC, reps=1, hw_loop=False):
    """Build the per-core Bass module. nb = blocks per core (small for sim)."""
    import concourse.bass as bass
    import concourse.mybir as mybir
    import concourse.tile as tile

    f32, bf16 = mybir.dt.float32, mybir.dt.bfloat16
    AF = mybir.ActivationFunctionType
    ALU = mybir.AluOpType
    nn = nb * NPB                     # nodes this build
    nts = min(512, nn)                # node tile size
    nt = nn // nts                    # node tiles

    nc = bass.Bass()
    h0T_d = nc.dram_tensor("h0T", [2, 128, nn], f32, kind="ExternalInput")
    crg_d = nc.dram_tensor("crg", [nb, 128, EPB], bf16, kind="ExternalInput")
    we2_d = nc.dram_tensor("we2", [128, L * 2 * 2 * 128], bf16, kind="ExternalInput")
    w1ab_d = nc.dram_tensor("w1ab", [128, L * 2 * 2 * 256], bf16, kind="ExternalInput")
    w1dp_d = nc.dram_tensor("w1dp", [64, L * 256], bf16, kind="ExternalInput")
    wn1a_d = nc.dram_tensor("wn1a", [128, L * 2 * 2 * 128], bf16, kind="ExternalInput")
    wn1b_d = nc.dram_tensor("wn1b", [128, L * 2 * 2 * 128], bf16, kind="ExternalInput")
    wn2_d = nc.dram_tensor("wn2", [128, L * 2 * 2 * 128], bf16, kind="ExternalInput")
    bnrow_d = nc.dram_tensor("bnrow", [1, L * 2 * 2 * 128], bf16, kind="ExternalInput")
    be12_d = nc.dram_tensor("be12", [128, 4 * L * 2], f32, kind="ExternalInput")
    out_d = nc.dram_tensor("poolT", [2, 128, nb], f32, kind="ExternalOutput")

    with tile.TileContext(nc) as tc:
        with (
            tc.tile_pool(name="const", bufs=1) as csp,
            tc.tile_pool(name="crgp", bufs=1) as crgp,
            tc.tile_pool(name="state", bufs=1) as stp,
            tc.tile_pool(name="comb", bufs=4) as combp,
            tc.tile_pool(name="m1p", bufs=3) as m1p,
            tc.tile_pool(name="m2p", bufs=6) as m2p,
            tc.tile_pool(name="ps", bufs=8, space="PSUM") as psp,
        ):
            # ---- resident constants
            we2_s = csp.tile([128, L * 2 * 2 * 128], bf16, name="we2_s")
            nc.sync.dma_start(we2_s[:], we2_d[:])
            w1ab_s = csp.tile([128, L * 2 * 2 * 256], bf16, name="w1ab_s")
            nc.sync.dma_start(w1ab_s[:], w1ab_d[:])
            w1dp_s = csp.tile([128, L * 256], bf16, name="w1dp_s")
            nc.sync.dma_start(w1dp_s[64:128, :], w1dp_d[:])
            wn1a_s = csp.tile([128, L * 2 * 2 * 128], bf16, name="wn1a_s")
            nc.sync.dma_start(wn1a_s[:], wn1a_d[:])
            wn1b_s = csp.tile([128, L * 2 * 2 * 128], bf16, name="wn1b_s")
            nc.sync.dma_start(wn1b_s[:], wn1b_d[:])
            wn2_s = csp.tile([128, L * 2 * 2 * 128], bf16, name="wn2_s")
            nc.sync.dma_start(wn2_s[:], wn2_d[:])
            bnrow_s = csp.tile([128, L * 2 * 2 * 128], bf16, name="bnrow_s")
            nc.sync.dma_start(bnrow_s[0:1, :], bnrow_d[:])
            be12_s = csp.tile([128, 4 * L * 2], f32, name="be12_s")
            nc.sync.dma_start(be12_s[:], be12_d[:])
            ones_s = csp.tile([128, 512], bf16, name="ones_s")
            nc.gpsimd.memset(ones_s[0:1, :], 1.0)

            def we2_ap(l, kc, mc):
                o = ((l * 2 + kc) * 2 + mc) * 128
                return we2_s[:, o:o + 128]

            def w1ab_ap(l, s, kc):
                o = ((l * 2 + s) * 2 + kc) * 256
                return w1ab_s[:, o:o + 256]

            def wfam_ap(t, l, kc, mc):
                o = ((l * 2 + kc) * 2 + mc) * 128
                return t[:, o:o + 128]

            def bnrow_ap(l, j, mc):
                o = ((l * 2 + j) * 2 + mc) * 128
                return bnrow_s[0:1, o:o + 128]

            def be_ap(j, l, mc):
                o = (j * L + l) * 2 + mc
                return be12_s[:, o:o + 1]

            # ---- CRG resident
            crg_s = []
            for b in range(nb):
                t = crgp.tile([128, EPB], bf16, name=f"crg{b}", tag=f"crg{b}")
                nc.sync.dma_start(t[:], crg_d[b])
                crg_s.append(t)

            # ---- state
            hT, hbf, aggT, aggbf, n1bf = [], [], [], [], []
            for c in range(2):
                t = stp.tile([128, nn], f32, name=f"hT{c}", tag=f"hT{c}")
                hT.append(t)
                hbf.append(stp.tile([128, nn], bf16, name=f"hbf{c}", tag=f"hbf{c}"))
                aggT.append(stp.tile([128, nn], f32, name=f"aggT{c}", tag=f"aggT{c}"))
                aggbf.append(stp.tile([128, nn], bf16, name=f"aggbf{c}", tag=f"agb{c}"))
                n1bf.append(stp.tile([128, nn], bf16, name=f"n1bf{c}", tag=f"n1b{c}"))

            import contextlib
            loop_ctx = (tc.For_i(0, reps, 1) if hw_loop
                        else contextlib.nullcontext())
            rep_range = range(1 if hw_loop else reps)
            with loop_ctx:
             for rep in rep_range:
              for c in range(2):
                nc.sync.dma_start(hT[c][:], h0T_d[c])
              for l in range(L):
                for c in range(2):
                    nc.gpsimd.tensor_copy(hbf[c][:], hT[c][:])
                # ---------------- edge phase, per block
                for b in range(nb):
                    ps_ab = psp.tile([128, 512], f32, tag="ps", name=f"ab{l}_{b}")
                    for sel, pos0 in ((0, 0), (1, 32)):
                        for kc in range(2):
                            nc.tensor.matmul(ps_ab[pos0:pos0 + 32, 0:256],
                                             lhsT=hbf[kc][:, b * NPB:(b + 1) * NPB],
                                             rhs=w1ab_ap(l, sel, kc),
                                             start=(kc == 0), stop=(kc == 1),
                                             tile_position=(0, pos0))
                    comb = combp.tile([128, 256], bf16, tag="comb", name=f"cb{l}_{b}")
                    nc.scalar.copy(comb[0:64, :], ps_ab[0:64, 0:256])
                    nc.gpsimd.tensor_copy(comb[64:128, :],
                                          w1dp_s[64:128, l * 256:(l + 1) * 256])
                    m1t = [m1p.tile([128, EPB], bf16, tag=f"m1_{kc}",
                                    name=f"m1_{l}_{b}_{kc}") for kc in range(2)]
                    for mc in range(2):
                        for h in range(2):
                            ps1 = psp.tile([128, 512], f32, tag="ps",
                                           name=f"p1_{l}_{b}_{mc}_{h}")
                            nc.tensor.matmul(ps1[:, 0:HALF],
                                             lhsT=comb[:, mc * 128:(mc + 1) * 128],
                                             rhs=crg_s[b][:, h * HALF:(h + 1) * HALF],
                                             start=True, stop=True)
                            nc.scalar.activation(m1t[mc][:, h * HALF:(h + 1) * HALF],
                                                 ps1[:, 0:HALF], AF.Relu,
                                                 bias=be_ap(0, l, mc))
                    for mc in range(2):
                        for h in range(2):
                            ps2 = psp.tile([128, 512], f32, tag="ps",
                                           name=f"p2_{l}_{b}_{mc}_{h}")
                            for kc in range(2):
                                nc.tensor.matmul(
                                    ps2[:, 0:HALF],
                                    lhsT=we2_ap(l, kc, mc),
                                    rhs=m1t[kc][:, h * HALF:(h + 1) * HALF],
                                    start=(kc == 0), stop=(kc == 1))
                            m2t = m2p.tile([128, HALF], bf16, tag="m2",
                                           name=f"m2_{l}_{b}_{mc}_{h}")
                            if (b * 4 + mc * 2 + h) % 2:
                                nc.scalar.activation(m2t[:], ps2[:, 0:HALF], AF.Relu,
                                                     bias=be_ap(1, l, mc))
                            else:
                                # relu(x + b) == max(x, -b) + b
                                nc.vector.scalar_tensor_tensor(
                                    m2t[:], ps2[:, 0:HALF], be_ap(2, l, mc),
                                    be_ap(1, l, mc).to_broadcast([128, HALF]),
                                    op0=ALU.max, op1=ALU.add)
                            nc.vector.tensor_reduce(
                                aggT[mc][:, b * NPB + h * 16: b * NPB + (h + 1) * 16],
                                m2t[:].rearrange("p (n k) -> p n k", k=K),
                                axis=mybir.AxisListType.X, op=ALU.add)
                # ---------------- node phase
                for c in range(2):
                    nc.gpsimd.tensor_copy(aggbf[c][:], aggT[c][:])
                for mc in range(2):
                    for t in range(nt):
                        sl = slice(t * nts, (t + 1) * nts)
                        psn = psp.tile([128, 512], f32, tag="ps",
                                       name=f"n1_{l}_{mc}_{t}")
                        nc.tensor.matmul(psn[:, 0:nts], lhsT=bnrow_ap(l, 0, mc),
                                         rhs=ones_s[0:1, 0:nts], start=True, stop=False)
                        for kc in range(2):
                            nc.tensor.matmul(psn[:, 0:nts],
                                             lhsT=wfam_ap(wn1a_s, l, kc, mc),
                                             rhs=hbf[kc][:, sl], start=False, stop=False)
                            nc.tensor.matmul(psn[:, 0:nts],
                                             lhsT=wfam_ap(wn1b_s, l, kc, mc),
                                             rhs=aggbf[kc][:, sl], start=False,
                                             stop=(kc == 1))
                        nc.scalar.activation(n1bf[mc][:, sl], psn[:, 0:nts], AF.Relu)
                for mc in range(2):
                    for t in range(nt):
                        sl = slice(t * nts, (t + 1) * nts)
                        pso = psp.tile([128, 512], f32, tag="ps",
                                       name=f"n2_{l}_{mc}_{t}")
                        nc.tensor.matmul(pso[:, 0:nts], lhsT=bnrow_ap(l, 1, mc),
                                         rhs=ones_s[0:1, 0:nts], start=True, stop=False)
                        for kc in range(2):
                            nc.tensor.matmul(pso[:, 0:nts],
                                             lhsT=wfam_ap(wn2_s, l, kc, mc),
                                             rhs=n1bf[kc][:, sl], start=False,
                                             stop=(kc == 1))
                        nc.vector.scalar_tensor_tensor(
                            hT[mc][:, sl], hT[mc][:, sl], 2.0, pso[:, 0:nts],
                            op0=ALU.mult, op1=ALU.add)
              # ---------------- pooling
              for mc in range(2):
                pool_t = stp.tile([128, nb], f32, tag=f"pool{mc}", name=f"pool{mc}")
                nc.vector.tensor_reduce(pool_t[:],
                                        hT[mc][:].rearrange("p (n k) -> p n k", k=NPB),
                                        axis=mybir.AxisListType.X, op=ALU.add)
                nc.scalar.mul(pool_t[:], pool_t[:], 1.0 / NPB)
                nc.sync.dma_start(out_d[mc], pool_t[:])
    return nc


# --------------------------------------------------- numpy model of the math

def numpy_model(ins, nb=BPC, cores=None):
    """Replicate the device math (incl. bf16 rounding) for validation.
    ins: list of per-core input dicts (from host_prep). Returns [sum_nb*NCORES? , 256]."""
    outs = []
    for m in (ins if cores is None else [ins[c] for c in cores]):
        h = np.asarray(m["h0T"], np.float32).reshape(256, -1)[:, :nb * NPB]  # [256, nn]
        crg = np.asarray(m["crg"], np.float32)[:nb]
        L4 = L
        we2 = np.asarray(m["we2"], np.float32).reshape(128, L4, 2, 2, 128).transpose(1, 2, 3, 0, 4)
        w1ab = np.asarray(m["w1ab"], np.float32).reshape(128, L4, 2, 2, 256).transpose(1, 2, 3, 0, 4)
        w1dp = np.asarray(m["w1dp"], np.float32).reshape(64, L4, 256).transpose(1, 0, 2)
        wn1a = np.asarray(m["wn1a"], np.float32).reshape(128, L4, 2, 2, 128).transpose(1, 2, 3, 0, 4)
        wn1b = np.asarray(m["wn1b"], np.float32).reshape(128, L4, 2, 2, 128).transpose(1, 2, 3, 0, 4)
        wn2 = np.asarray(m["wn2"], np.float32).reshape(128, L4, 2, 2, 128).transpose(1, 2, 3, 0, 4)
        bnrow = np.asarray(m["bnrow"], np.float32).reshape(1, L4, 2, 2, 128).transpose(1, 2, 3, 0, 4)
        be12 = np.asarray(m["be12"], np.float32)
        nn = nb * NPB

        def b16(x):
            return x.astype(BF16).astype(np.float32)

        def blk(w):  # [kc, mc, 128, 128] -> [256, 256]
            return np.concatenate(
                [np.concatenate([w[kc_, mc_] for mc_ in range(2)], axis=1)
                 for kc_ in range(2)], axis=0)

        for l in range(L):
            hb = b16(h)                                    # [256, nn]
            # hAB per block
            W1b = np.concatenate([w1ab[l, 0, kc_] for kc_ in range(2)], axis=0)
            W1a = np.concatenate([w1ab[l, 1, kc_] for kc_ in range(2)], axis=0)
            be1 = np.concatenate([be12[:, (0 * L + l) * 2 + mc_] for mc_ in range(2)])
            be2 = np.concatenate([be12[:, (1 * L + l) * 2 + mc_] for mc_ in range(2)])
            agg = np.zeros((256, nn), np.float32)
            for b in range(nb):
                hs = hb[:, b * NPB:(b + 1) * NPB]          # [256, 32]
                hB = b16(hs.T @ W1b)                       # [32, 256] evicted bf16
                hA = b16(hs.T @ W1a)
                combined = np.concatenate([hB, hA, w1dp[l]], axis=0)  # [128, 256]
                pre1 = combined.T @ crg[b]                 # [256, EPB]
                m1 = b16(np.maximum(pre1 + be1[:, None], 0.0))
                W2 = blk(we2[l])
                m2 = b16(np.maximum(W2.T @ m1 + be2[:, None], 0.0))
                agg[:, b * NPB:(b + 1) * NPB] = (
                    m2.reshape(256, NPB, K).sum(axis=2))
            aggb = b16(agg)
            N1a, N1b_, N2 = blk(wn1a[l]), blk(wn1b[l]), blk(wn2[l])
            bn1 = bnrow[l, 0].reshape(256)
            bn2 = bnrow[l, 1].reshape(256)
            n1 = b16(np.maximum(N1a.T @ hb + N1b_.T @ aggb + bn1[:, None], 0.0))
            out = N2.T @ n1 + bn2[:, None]
            h = 2.0 * h + out
        pooled = h.reshape(256, nb, NPB).mean(axis=2)       # [256, nb]
        outs.append(pooled.T)
    return np.concatenate(outs, axis=0)


# --------------------------------------------------------------- builder v2
# m2 in normal layout (edges on partitions); segment-sum as PE matmuls with
# constant Ssel matrices; agg evicted straight to bf16.

def build_nc_v2(nb=BPC, reps=1, hw_loop=False, be2_mm=False,
                m1_dve_of_8=2, m2_dve_of_8=5, comb_dve_of_8=0, agg_dve_of_8=0):
    import contextlib
    import concourse.bass as bass
    import concourse.mybir as mybir
    import concourse.tile as tile

    f32, bf16 = mybir.dt.float32, mybir.dt.bfloat16
    AF = mybir.ActivationFunctionType
    ALU = mybir.AluOpType
    nn = nb * NPB
    nts = min(512, nn)
    nt = nn // nts

    nc = bass.Bass()
    h0T_d = nc.dram_tensor("h0T", [2, 128, nn], f32, kind="ExternalInput")
    crg_d = nc.dram_tensor("crg", [nb, 128, EPB], bf16, kind="ExternalInput")
    we2r_d = nc.dram_tensor("we2r", [128, L * 2 * 256], bf16, kind="ExternalInput")
    w1ab_d = nc.dram_tensor("w1ab", [128, L * 2 * 2 * 256], bf16, kind="ExternalInput")
    w1dp_d = nc.dram_tensor("w1dp", [64, L * 256], bf16, kind="ExternalInput")
    wn1a_d = nc.dram_tensor("wn1a", [128, L * 2 * 2 * 128], bf16, kind="ExternalInput")
    wn1b_d = nc.dram_tensor("wn1b", [128, L * 2 * 2 * 128], bf16, kind="ExternalInput")
    wn2_d = nc.dram_tensor("wn2", [128, L * 2 * 2 * 128], bf16, kind="ExternalInput")
    bnrow_d = nc.dram_tensor("bnrow", [1, L * 2 * 2 * 128], bf16, kind="ExternalInput")
    be12_d = nc.dram_tensor("be12", [128, 4 * L * 2], f32, kind="ExternalInput")
    be2row_d = nc.dram_tensor("be2row", [1, L * 512], bf16, kind="ExternalInput")
    ssel_d = nc.dram_tensor("ssel", [128, 5 * NPB], bf16, kind="ExternalInput")
    out_d = nc.dram_tensor("poolT", [2, 128, nb], f32, kind="ExternalOutput")

    with tile.TileContext(nc) as tc:
        with (
            tc.tile_pool(name="const", bufs=1) as csp,
            tc.tile_pool(name="crgp", bufs=1) as crgp,
            tc.tile_pool(name="state", bufs=1) as stp,
            tc.tile_pool(name="comb", bufs=1) as combp,
            tc.tile_pool(name="m1p", bufs=5) as m1p,
            tc.tile_pool(name="m2p", bufs=14) as m2p,
            tc.tile_pool(name="ps", bufs=7, space="PSUM") as psp,
            tc.tile_pool(name="psagg", bufs=1, space="PSUM") as psaggp,
        ):
            we2r_s = csp.tile([128, L * 2 * 256], bf16, name="we2r_s")
            nc.sync.dma_start(we2r_s[:], we2r_d[:])
            w1ab_s = csp.tile([128, L * 2 * 2 * 256], bf16, name="w1ab_s")
            nc.sync.dma_start(w1ab_s[:], w1ab_d[:])
            w1dp_s = csp.tile([128, L * 256], bf16, name="w1dp_s")
            nc.sync.dma_start(w1dp_s[64:128, :], w1dp_d[:])
            wn1a_s = csp.tile([128, L * 2 * 2 * 128], bf16, name="wn1a_s")
            nc.sync.dma_start(wn1a_s[:], wn1a_d[:])
            wn1b_s = csp.tile([128, L * 2 * 2 * 128], bf16, name="wn1b_s")
            nc.sync.dma_start(wn1b_s[:], wn1b_d[:])
            wn2_s = csp.tile([128, L * 2 * 2 * 128], bf16, name="wn2_s")
            nc.sync.dma_start(wn2_s[:], wn2_d[:])
            bnrow_s = csp.tile([128, L * 2 * 2 * 128], bf16, name="bnrow_s")
            nc.sync.dma_start(bnrow_s[0:1, :], bnrow_d[:])
            be12_s = csp.tile([128, 4 * L * 2], f32, name="be12_s")
            nc.sync.dma_start(be12_s[:], be12_d[:])
            be2row_s = csp.tile([128, L * 512], bf16, name="be2row_s")
            nc.sync.dma_start(be2row_s[0:1, :], be2row_d[:])
            ssel_s = csp.tile([128, 5 * NPB], bf16, name="ssel_s")
            nc.sync.dma_start(ssel_s[:], ssel_d[:])
            ones_s = csp.tile([128, 512], bf16, name="ones_s")
            nc.gpsimd.memset(ones_s[0:1, :], 1.0)
            zcol_s = csp.tile([128, 1], f32, name="zcol_s")
            nc.gpsimd.memset(zcol_s[:], 0.0)

            def we2r_ap(l, kc):
                o = (l * 2 + kc) * 256
                return we2r_s[:, o:o + 256]

            def w1ab_ap(l, sel, kc):
                o = ((l * 2 + sel) * 2 + kc) * 256
                return w1ab_s[:, o:o + 256]

            def wfam_ap(t, l, kc, mc):
                o = ((l * 2 + kc) * 2 + mc) * 128
                return t[:, o:o + 128]

            def bnrow_ap(l, j, mc):
                o = ((l * 2 + j) * 2 + mc) * 128
                return bnrow_s[0:1, o:o + 128]

            def be_ap(j, l, mc):
                o = (j * L + l) * 2 + mc
                return be12_s[:, o:o + 1]

            hT, hbf, aggbf, n1bf = [], [], [], []
            for c in range(2):
                hT.append(stp.tile([128, nn], f32, name=f"hT{c}", tag=f"hT{c}"))
                hbf.append(stp.tile([128, nn], bf16, name=f"hbf{c}", tag=f"hbf{c}"))
                aggbf.append(stp.tile([128, nn], bf16, name=f"agb{c}", tag=f"agb{c}"))
                n1bf.append(stp.tile([128, nn], bf16, name=f"n1b{c}", tag=f"n1b{c}"))

            if not hw_loop:
                for c in range(2):
                    for t in range(nt):
                        sl = slice(t * nts, (t + 1) * nts)
                        nc.sync.dma_start(hT[c][:, sl], h0T_d[c][:, sl])
                        nc.gpsimd.tensor_copy(hbf[c][:, sl], hT[c][:, sl])

            crg_s = []
            for b in range(nb):
                t = crgp.tile([128, EPB], bf16, name=f"crg{b}", tag=f"crg{b}")
                nc.sync.dma_start(t[:], crg_d[b])
                crg_s.append(t)


            comb_tiles = [
                [combp.tile([128, 256], bf16, tag=f"comb{l}_{i}",
                            name=f"comb{l}_{i}") for i in range(min(4, nb))]
                for l in range(L)]

            evict_i = [0]

            def evict(out_ap, ps_ap, relu, bias_ap, dve_of_8):
                """PSUM->SBUF eviction on ACT or DVE (round-robin)."""
                use_dve = (evict_i[0] % 8) < dve_of_8
                evict_i[0] += 1
                if relu:
                    if use_dve and bias_ap is None:
                        nc.vector.scalar_tensor_tensor(
                            out_ap, ps_ap, 0.0,
                            zcol_s[:, 0:1].to_broadcast(
                                [out_ap.shape[0], out_ap.free_size()]),
                            op0=ALU.max, op1=ALU.add)
                    elif use_dve:
                        # relu(x + b) == max(x, -b) + b ; bias_ap=(be, neg_be)
                        be, nbe = bias_ap
                        nc.vector.scalar_tensor_tensor(
                            out_ap, ps_ap, nbe,
                            be.to_broadcast([out_ap.shape[0], out_ap.free_size()]),
                            op0=ALU.max, op1=ALU.add)
                    else:
                        nc.scalar.activation(out_ap, ps_ap, AF.Relu,
                                             bias=(bias_ap[0] if bias_ap else 0.0))
                else:
                    if use_dve:
                        nc.vector.tensor_copy(out_ap, ps_ap)
                    else:
                        nc.scalar.copy(out_ap, ps_ap)

            loop_ctx = (tc.For_i(0, reps, 1) if hw_loop else contextlib.nullcontext())
            rep_range = range(1 if hw_loop else reps)
            with loop_ctx:
             for rep in rep_range:
              if hw_loop or rep > 0:
                for c in range(2):
                    for t in range(nt):
                        sl = slice(t * nts, (t + 1) * nts)
                        nc.sync.dma_start(hT[c][:, sl], h0T_d[c][:, sl])
                        nc.gpsimd.tensor_copy(hbf[c][:, sl], hT[c][:, sl])
              for l in range(L):
                for i in range(min(4, nb)):
                    nc.gpsimd.tensor_copy(
                        comb_tiles[l][i][64:128, :],
                        w1dp_s[64:128, l * 256:(l + 1) * 256])
                for g in range(nb // 4):
                    agg_ps = psaggp.tile([128, 256], f32, tag="agg",
                                         name=f"agg{l}_{g}")
                    # ---- pass A: hA/hB for 4 blocks
                    for bi in range(4):
                        b = g * 4 + bi
                        ps_ab = psp.tile([128, 512], f32, tag="ps",
                                         name=f"ab{l}_{b}")
                        for kc in range(2):
                            for sel, pos0 in ((0, 0), (1, 32)):
                                nc.tensor.matmul(
                                    ps_ab[pos0:pos0 + 32, 0:256],
                                    lhsT=hbf[kc][:, b * NPB:(b + 1) * NPB],
                                    rhs=w1ab_ap(l, sel, kc),
                                    start=(kc == 0), stop=(kc == 1),
                                    tile_position=(0, pos0),
                                    skip_group_check=True)
                        comb = comb_tiles[l][b % 4]
                        evict(comb[0:64, :], ps_ab[0:64, 0:256], False, None,
                              comb_dve_of_8)
                    # ---- pass B: edge MLP layer 1 (transposed out)
                    m1ts = {}
                    for bi in range(4):
                        b = g * 4 + bi
                        comb = comb_tiles[l][b % 4]
                        m1t = [m1p.tile([128, EPB], bf16, tag=f"m1_{kc}",
                                        name=f"m1_{l}_{b}_{kc}") for kc in range(2)]
                        m1ts[bi] = m1t
                        for mc in range(2):
                            for h in range(2):
                                ps1 = psp.tile([128, 512], f32, tag="ps",
                                               name=f"p1_{l}_{b}_{mc}_{h}")
                                nc.tensor.matmul(
                                    ps1[:, 0:HALF],
                                    lhsT=comb[:, mc * 128:(mc + 1) * 128],
                                    rhs=crg_s[b][:, h * HALF:(h + 1) * HALF],
                                    start=True, stop=True)
                                evict(m1t[mc][:, h * HALF:(h + 1) * HALF],
                                      ps1[:, 0:HALF], True,
                                      (be_ap(0, l, mc), be_ap(3, l, mc)),
                                      m1_dve_of_8)
                    # ---- pass C: edge MLP layer 2 (normal out)
                    m2ss = {}
                    for bi in range(4):
                        b = g * 4 + bi
                        m1t = m1ts[bi]
                        m2sbs = []
                        for p in range(3):
                            ecs = (2 * p, 2 * p + 1) if p < 2 else (4,)
                            w = 256 * len(ecs)
                            ps2 = psp.tile([128, 512], f32, tag="ps",
                                           name=f"p2_{l}_{b}_{p}")
                            for j, ec in enumerate(ecs):
                                if be2_mm:
                                    nc.tensor.matmul(
                                        ps2[:, j * 256:(j + 1) * 256],
                                        lhsT=ones_s[0:1, 0:128],
                                        rhs=be2row_s[0:1, l * 512:l * 512 + 256],
                                        start=True, stop=False)
                                for kc in range(2):
                                    nc.tensor.matmul(
                                        ps2[:, j * 256:(j + 1) * 256],
                                        lhsT=m1t[kc][:, ec * 128:(ec + 1) * 128],
                                        rhs=we2r_ap(l, kc),
                                        start=(kc == 0 and not be2_mm),
                                        stop=(kc == 1))
                            m2sb = m2p.tile([128, 512], bf16, tag="m2",
                                            name=f"m2_{l}_{b}_{p}")
                            evict(m2sb[:, 0:w], ps2[:, 0:w], True, None,
                                  m2_dve_of_8)
                            m2sbs.append(m2sb)
                        m2ss[bi] = m2sbs
                    # ---- pass D: PE segment-sum into agg psum
                    for bi in range(4):
                        m2sbs = m2ss[bi]
                        for mc in range(2):
                            for ec in range(5):
                                p, j = divmod(ec, 2)
                                nc.tensor.matmul(
                                    agg_ps[:, mc * 128 + bi * 32:
                                           mc * 128 + bi * 32 + 32],
                                    lhsT=m2sbs[p][:, j * 256 + mc * 128:
                                                  j * 256 + (mc + 1) * 128],
                                    rhs=ssel_s[:, ec * NPB:(ec + 1) * NPB],
                                    start=(ec == 0), stop=(ec == 4))
                    # ---- agg eviction for this 4-block group (bf16 cast)
                    for mc in range(2):
                        evict(aggbf[mc][:, g * 128:(g + 1) * 128],
                              agg_ps[:, mc * 128:(mc + 1) * 128], False, None,
                              agg_dve_of_8)
                # ---------------- node phase
                for mc in range(2):
                    for t in range(nt):
                        sl = slice(t * nts, (t + 1) * nts)
                        psn = psp.tile([128, 512], f32, tag="ps",
                                       name=f"n1_{l}_{mc}_{t}")
                        nc.tensor.matmul(psn[:, 0:nts], lhsT=bnrow_ap(l, 0, mc),
                                         rhs=ones_s[0:1, 0:nts],
                                         start=True, stop=False)
                        for kc in range(2):
                            nc.tensor.matmul(psn[:, 0:nts],
                                             lhsT=wfam_ap(wn1a_s, l, kc, mc),
                                             rhs=hbf[kc][:, sl],
                                             start=False, stop=False)
                            nc.tensor.matmul(psn[:, 0:nts],
                                             lhsT=wfam_ap(wn1b_s, l, kc, mc),
                                             rhs=aggbf[kc][:, sl],
                                             start=False, stop=(kc == 1))
                        nc.scalar.activation(n1bf[mc][:, sl], psn[:, 0:nts], AF.Relu)
                for mc in range(2):
                    for t in range(nt):
                        sl = slice(t * nts, (t + 1) * nts)
                        pso = psp.tile([128, 512], f32, tag="ps",
                                       name=f"n2_{l}_{mc}_{t}")
                        nc.tensor.matmul(pso[:, 0:nts], lhsT=bnrow_ap(l, 1, mc),
                                         rhs=ones_s[0:1, 0:nts],
                                         start=True, stop=False)
                        for kc in range(2):
                            nc.tensor.matmul(pso[:, 0:nts],
                                             lhsT=wfam_ap(wn2_s, l, kc, mc),
                                             rhs=n1bf[kc][:, sl],
                                             start=False, stop=(kc == 1))
                        nc.vector.scalar_tensor_tensor(
                            hT[mc][:, sl], hT[mc][:, sl], 2.0, pso[:, 0:nts],
                            op0=ALU.mult, op1=ALU.add)
                        if l + 1 < L:
                            nc.gpsimd.tensor_copy(hbf[mc][:, sl], hT[mc][:, sl])
              # ---------------- pooling
              for mc in range(2):
                pool_t = stp.tile([128, nb], f32, tag=f"pool{mc}", name=f"pool{mc}")
                nc.vector.tensor_reduce(pool_t[:],
                                        hT[mc][:].rearrange("p (n k) -> p n k", k=NPB),
                                        axis=mybir.AxisListType.X, op=ALU.add)
                nc.scalar.mul(pool_t[:], pool_t[:], 1.0 / NPB)
                nc.sync.dma_start(out_d[mc], pool_t[:])
    return nc


# --------------------------------------------------------------- builder v3
# Edge MLP layer 2 as fp8 DoubleRow matmuls (K=256 in one pass, stationary
# We2), m2 produced transposed; segment-sum as DVE grouped reduce (k=20)
# off the PE. Bias matmuls removed (bn1 via ACT bias; bn2 must be zero).
# Per-layer power-of-2 scaling keeps fp8 operands in range:
#   comb/w1dp/be1 scaled by CM[l]; We2 scaled by CW; m2 evict rescales
#   by 1/(CM[l]*CW) via the ACT scale arg (exact, powers of two).

CM_DR = [32.0, 16.0, 4.0, 1.0]   # m1 fp8 scale, used only on DR layers
CW = 8.0
FP8MAX = 240.0
DR_LAYERS = ()                    # layers running edge-MLP2 as fp8 DoubleRow


def _cm(l, dr_layers):
    return CM_DR[l] if l in dr_layers else 1.0


def host_prep_v3(np_inputs, dr_layers=DR_LAYERS):
    """Extra per-core tensors for the v3 builder (on top of host_prep)."""
    We2 = np.asarray(np_inputs["We2"], np.float32)
    We1 = np.asarray(np_inputs["We1"], np.float32)
    be1 = np.asarray(np_inputs["be1"], np.float32)
    be2 = np.asarray(np_inputs["be2"], np.float32)
    bn1 = np.asarray(np_inputs["bn1"], np.float32)
    E4 = ml_dtypes.float8_e4m3fn

    # We2 DoubleRow pack [128, L*2*2*128] fp8; slice (l, mc) = [128, 2(kc), 128]
    w = np.clip(We2 * CW, -FP8MAX, FP8MAX)
    we2dr = np.ascontiguousarray(
        w.reshape(L, 2, 128, 2, 128).transpose(2, 0, 3, 1, 4).reshape(128, -1)
    ).astype(E4)

    # w1dp scaled per layer [64, L*256] bf16
    cm = np.asarray([_cm(l, dr_layers) for l in range(L)], np.float32)
    w1dp = np.concatenate([We1[:, 512:513, :], We1[:, 513:576, :]], axis=1)
    w1dp = w1dp * cm[:, None, None]
    w1dpv3 = np.ascontiguousarray(
        w1dp.transpose(1, 0, 2).reshape(64, -1)).astype(BF16)

    # bias table [128, 5*L*2] f32; col = (j*L + l)*2 + mc
    # j: 0 = be1*cm, 1 = -be1*cm, 2 = be2, 3 = -be2, 4 = bn1
    bias = np.zeros((128, 5 * L * 2), np.float32)
    for l in range(L):
        rows = [be1[l] * cm[l], -be1[l] * cm[l], be2[l], -be2[l], bn1[l]]
        for j, r in enumerate(rows):
            for mc in range(2):
                bias[:, (j * L + l) * 2 + mc] = r[mc * 128:(mc + 1) * 128]

    return dict(we2dr=we2dr, w1dpv3=w1dpv3, biasv3=bias)


def build_nc_v3(nb=BPC, reps=1, hw_loop=False, be2_nz=False,
                dr_layers=DR_LAYERS,
                m1_pat="APDAPDAP", m2_pat="PADPADPA", comb_pat="PPAP",
                red_pat="D"):
    import contextlib
    import concourse.bass as bass
    import concourse.mybir as mybir
    import concourse.tile as tile

    f32, bf16 = mybir.dt.float32, mybir.dt.bfloat16
    fp8 = mybir.dt.float8e4
    AF = mybir.ActivationFunctionType
    ALU = mybir.AluOpType
    DR = mybir.MatmulPerfMode.DoubleRow
    nn = nb * NPB
    nts = min(512, nn)
    nt = nn // nts

    nc = bass.Bass()
    h0T_d = nc.dram_tensor("h0T", [2, 128, nn], f32, kind="ExternalInput")
    crg_d = nc.dram_tensor("crg", [nb, 128, EPB], bf16, kind="ExternalInput")
    we2_d = nc.dram_tensor("we2", [128, L * 2 * 2 * 128], bf16,
                           kind="ExternalInput")
    we2dr_d = nc.dram_tensor("we2dr", [128, L * 2 * 2 * 128], fp8,
                             kind="ExternalInput")
    w1ab_d = nc.dram_tensor("w1ab", [128, L * 2 * 2 * 256], bf16,
                            kind="ExternalInput")
    w1dpv3_d = nc.dram_tensor("w1dpv3", [64, L * 256], bf16, kind="ExternalInput")
    wn1a_d = nc.dram_tensor("wn1a", [128, L * 2 * 2 * 128], bf16,
                            kind="ExternalInput")
    wn1b_d = nc.dram_tensor("wn1b", [128, L * 2 * 2 * 128], bf16,
                            kind="ExternalInput")
    wn2_d = nc.dram_tensor("wn2", [128, L * 2 * 2 * 128], bf16,
                           kind="ExternalInput")
    biasv3_d = nc.dram_tensor("biasv3", [128, 5 * L * 2], f32,
                              kind="ExternalInput")
    out_d = nc.dram_tensor("poolT", [2, 128, nb], f32, kind="ExternalOutput")

    with tile.TileContext(nc) as tc:
        with (
            tc.tile_pool(name="const", bufs=1) as csp,
            tc.tile_pool(name="crgp", bufs=1) as crgp,
            tc.tile_pool(name="state", bufs=1) as stp,
            tc.tile_pool(name="comb", bufs=1) as combp,
            tc.tile_pool(name="m1p", bufs=3) as m1p,
            tc.tile_pool(name="m2p", bufs=8) as m2p,
            tc.tile_pool(name="ps", bufs=1, space="PSUM") as psp,
        ):
            we2_s = csp.tile([128, L * 2 * 2 * 128], bf16, name="we2_s")
            nc.sync.dma_start(we2_s[:], we2_d[:])
            we2dr_s = csp.tile([128, L * 2 * 2 * 128], fp8, name="we2dr_s")
            nc.sync.dma_start(we2dr_s[:], we2dr_d[:])
            w1ab_s = csp.tile([128, L * 2 * 2 * 256], bf16, name="w1ab_s")
            nc.sync.dma_start(w1ab_s[:], w1ab_d[:])
            w1dp_s = csp.tile([128, L * 256], bf16, name="w1dp_s")
            nc.sync.dma_start(w1dp_s[64:128, :], w1dpv3_d[:])
            wn1a_s = csp.tile([128, L * 2 * 2 * 128], bf16, name="wn1a_s")
            nc.sync.dma_start(wn1a_s[:], wn1a_d[:])
            wn1b_s = csp.tile([128, L * 2 * 2 * 128], bf16, name="wn1b_s")
            nc.sync.dma_start(wn1b_s[:], wn1b_d[:])
            wn2_s = csp.tile([128, L * 2 * 2 * 128], bf16, name="wn2_s")
            nc.sync.dma_start(wn2_s[:], wn2_d[:])
            bias_s = csp.tile([128, 5 * L * 2], f32, name="bias_s")
            nc.sync.dma_start(bias_s[:], biasv3_d[:])
            zcol_s = csp.tile([128, 1], f32, name="zcol_s")
            nc.gpsimd.memset(zcol_s[:], 0.0)

            def w1ab_ap(l, sel, kc):
                o = ((l * 2 + sel) * 2 + kc) * 256
                return w1ab_s[:, o:o + 256]

            def we2dr_ap(l, mc):
                o = (l * 2 + mc) * 256
                return we2dr_s[:, o:o + 256].rearrange("p (k m) -> p k m", k=2)

            def we2_ap(l, kc, mc):
                o = ((l * 2 + kc) * 2 + mc) * 128
                return we2_s[:, o:o + 128]

            def wfam_ap(t, l, kc, mc):
                o = ((l * 2 + kc) * 2 + mc) * 128
                return t[:, o:o + 128]

            def bv3(j, l, mc):
                o = (j * L + l) * 2 + mc
                return bias_s[:, o:o + 1]

            hT, hbf, aggbf, n1bf = [], [], [], []
            for c in range(2):
                hT.append(stp.tile([128, nn], f32, name=f"hT{c}", tag=f"hT{c}"))
                hbf.append(stp.tile([128, nn], bf16, name=f"hbf{c}", tag=f"hbf{c}"))
                aggbf.append(stp.tile([128, nn], bf16, name=f"agb{c}", tag=f"agb{c}"))
                n1bf.append(stp.tile([128, nn], bf16, name=f"n1b{c}", tag=f"n1b{c}"))

            if not hw_loop:
                for c in range(2):
                    for t in range(nt):
                        sl = slice(t * nts, (t + 1) * nts)
                        nc.sync.dma_start(hT[c][:, sl], h0T_d[c][:, sl])
                        nc.gpsimd.tensor_copy(hbf[c][:, sl], hT[c][:, sl])

            crg_s = []
            for b in range(nb):
                t = crgp.tile([128, EPB], bf16, name=f"crg{b}", tag=f"crg{b}")
                nc.sync.dma_start(t[:], crg_d[b])
                crg_s.append(t)

            comb_tiles = [
                [combp.tile([128, 256], bf16, tag=f"comb{l}_{i}",
                            name=f"comb{l}_{i}") for i in range(min(4, nb))]
                for l in range(L)]

            # engine dispatch: 'A' = ACT, 'D' = DVE, 'P' = Pool/gpsimd
            ev_i = {"m1": 0, "m2": 0, "comb": 0, "red": 0}
            pats = {"m1": m1_pat, "m2": m2_pat, "comb": comb_pat, "red": red_pat}

            def eng(kind):
                ch = pats[kind][ev_i[kind] % len(pats[kind])]
                ev_i[kind] += 1
                return ch

            def evict_comb(out_ap, ps_ap, scale):
                ch = eng("comb")
                if ch == "A":
                    nc.scalar.mul(out_ap, ps_ap, scale)
                elif ch == "D":
                    nc.vector.tensor_scalar_mul(out_ap, ps_ap, scale)
                else:
                    nc.gpsimd.tensor_scalar_mul(out_ap, ps_ap, scale)

            def evict_m1(out_ap, ps_ap, l, kc):
                ch = eng("m1")
                if ch == "A":
                    nc.scalar.activation(out_ap, ps_ap, AF.Relu,
                                         bias=bv3(0, l, kc))
                else:
                    e = nc.vector if ch == "D" else nc.gpsimd
                    e.scalar_tensor_tensor(
                        out_ap, ps_ap, bv3(1, l, kc),
                        bv3(0, l, kc).to_broadcast(
                            [out_ap.shape[0], out_ap.free_size()]),
                        op0=ALU.max, op1=ALU.add)

            def evict_m2(out_ap, ps_ap, l, mc, s2):
                ch = "A" if be2_nz else eng("m2")
                if ch == "A":
                    nc.scalar.activation(out_ap, ps_ap, AF.Relu,
                                         bias=bv3(2, l, mc), scale=s2)
                else:
                    e = nc.vector if ch == "D" else nc.gpsimd
                    e.tensor_scalar(out_ap, ps_ap, s2, 0.0,
                                    op0=ALU.mult, op1=ALU.max)

            import contextlib as _ctx
            loop_ctx = (tc.For_i(0, reps, 1) if hw_loop else _ctx.nullcontext())
            rep_range = range(1 if hw_loop else reps)
            with loop_ctx:
             for rep in rep_range:
              if hw_loop or rep > 0:
                for c in range(2):
                    for t in range(nt):
                        sl = slice(t * nts, (t + 1) * nts)
                        nc.sync.dma_start(hT[c][:, sl], h0T_d[c][:, sl])
                        nc.gpsimd.tensor_copy(hbf[c][:, sl], hT[c][:, sl])
              for l in range(L):
                is_dr = l in dr_layers
                s2 = 1.0 / (_cm(l, dr_layers) * CW) if is_dr else 1.0
                for i in range(min(4, nb)):
                    nc.gpsimd.tensor_copy(
                        comb_tiles[l][i][64:128, :],
                        w1dp_s[64:128, l * 256:(l + 1) * 256])

                def emit_A(b):
                    ps_ab = psp.tile([128, 512], f32, tag="psA", bufs=2,
                                     name=f"ab{l}_{b}")
                    for sel, pos0 in ((0, 0), (1, 32)):
                        for kc in range(2):
                            nc.tensor.matmul(
                                ps_ab[pos0:pos0 + 32, 0:256],
                                lhsT=hbf[kc][:, b * NPB:(b + 1) * NPB],
                                rhs=w1ab_ap(l, sel, kc),
                                start=(kc == 0), stop=(kc == 1),
                                tile_position=(0, pos0),
                                skip_group_check=True)
                    comb = comb_tiles[l][b % 4]
                    evict_comb(comb[0:64, :], ps_ab[0:64, 0:256],
                               _cm(l, dr_layers))

                def emit_B(b):
                    if is_dr:
                        m1t = m1p.tile([128, 2, EPB], fp8, tag="m1f",
                                       name=f"m1_{l}_{b}")
                        m1aps = [m1t[:, kc, :] for kc in range(2)]
                    else:
                        m1a = m1p.tile([128, EPB], bf16, tag="m1a",
                                       name=f"m1a_{l}_{b}")
                        m1b = m1p.tile([128, EPB], bf16, tag="m1b",
                                       name=f"m1b_{l}_{b}")
                        m1t = [m1a, m1b]
                        m1aps = m1t
                    comb = comb_tiles[l][b % 4]
                    for kc in range(2):
                        for h in range(2):
                            ps1 = psp.tile([128, 512], f32, tag="ps1", bufs=2,
                                           name=f"p1_{l}_{b}_{kc}_{h}")
                            nc.tensor.matmul(
                                ps1[:, 0:HALF],
                                lhsT=comb[:, kc * 128:(kc + 1) * 128],
                                rhs=crg_s[b][:, h * HALF:(h + 1) * HALF],
                                start=True, stop=True)
                            evict_m1(m1aps[kc][:, h * HALF:(h + 1) * HALF],
                                     ps1[:, 0:HALF], l, kc)
                    return m1t

                def emit_C(b, m1t):
                    for mc in range(2):
                        for h in range(2):
                            hs = slice(h * HALF, (h + 1) * HALF)
                            ps2 = psp.tile([128, 512], f32, tag="psm2", bufs=4,
                                           name=f"p2_{l}_{b}_{mc}_{h}")
                            if is_dr:
                                nc.tensor.matmul(
                                    ps2[:, 0:HALF], lhsT=we2dr_ap(l, mc),
                                    rhs=m1t[:, :, hs],
                                    start=True, stop=True, perf_mode=DR)
                            else:
                                for kc in range(2):
                                    nc.tensor.matmul(
                                        ps2[:, 0:HALF],
                                        lhsT=we2_ap(l, kc, mc),
                                        rhs=m1t[kc][:, hs],
                                        start=(kc == 0), stop=(kc == 1))
                            m2sb = m2p.tile([128, HALF], bf16, tag="m2",
                                            name=f"m2_{l}_{b}_{mc}_{h}")
                            evict_m2(m2sb[:], ps2[:, 0:HALF], l, mc, s2)
                            with nc.allow_low_precision("bf16 agg as v2"):
                                nc.vector.tensor_reduce(
                                    aggbf[mc][:, b * NPB + h * 16:
                                              b * NPB + (h + 1) * 16],
                                    m2sb[:].rearrange("p (n k) -> p n k", k=K),
                                    axis=mybir.AxisListType.X, op=ALU.add)

                # software pipeline: A leads by 1 block, C lags by 1 block
                emit_A(0)
                m1_prev = None
                for b in range(nb):
                    m1_cur = emit_B(b)
                    if b + 1 < nb:
                        emit_A(b + 1)
                    if m1_prev is not None:
                        emit_C(b - 1, m1_prev)
                    m1_prev = m1_cur
                emit_C(nb - 1, m1_prev)

                # ---------------- node phase
                for mc in range(2):
                    for t in range(nt):
                        sl = slice(t * nts, (t + 1) * nts)
                        psn = psp.tile([128, 512], f32, tag="ps1", bufs=2,
                                       name=f"n1_{l}_{mc}_{t}")
                        for kc in range(2):
                            nc.tensor.matmul(psn[:, 0:nts],
                                             lhsT=wfam_ap(wn1a_s, l, kc, mc),
                                             rhs=hbf[kc][:, sl],
                                             start=(kc == 0), stop=False)
                            nc.tensor.matmul(psn[:, 0:nts],
                                             lhsT=wfam_ap(wn1b_s, l, kc, mc),
                                             rhs=aggbf[kc][:, sl],
                                             start=False, stop=(kc == 1))
                        nc.scalar.activation(n1bf[mc][:, sl], psn[:, 0:nts],
                                             AF.Relu, bias=bv3(4, l, mc))
                for mc in range(2):
                    for t in range(nt):
                        sl = slice(t * nts, (t + 1) * nts)
                        pso = psp.tile([128, 512], f32, tag="psm2", bufs=4,
                                       name=f"n2_{l}_{mc}_{t}")
                        for kc in range(2):
                            nc.tensor.matmul(pso[:, 0:nts],
                                             lhsT=wfam_ap(wn2_s, l, kc, mc),
                                             rhs=n1bf[kc][:, sl],
                                             start=(kc == 0), stop=(kc == 1))
                        nc.vector.scalar_tensor_tensor(
                            hT[mc][:, sl], hT[mc][:, sl], 2.0, pso[:, 0:nts],
                            op0=ALU.mult, op1=ALU.add)
                        if l + 1 < L:
                            nc.gpsimd.tensor_copy(hbf[mc][:, sl], hT[mc][:, sl])
              # ---------------- pooling
              for mc in range(2):
                pool_t = stp.tile([128, nb], f32, tag=f"pool{mc}", name=f"pool{mc}")
                nc.vector.tensor_reduce(pool_t[:],
                                        hT[mc][:].rearrange("p (n k) -> p n k", k=NPB),
                                        axis=mybir.AxisListType.X, op=ALU.add)
                nc.scalar.mul(pool_t[:], pool_t[:], 1.0 / NPB)
                nc.sync.dma_start(out_d[mc], pool_t[:])
    return nc


# --------------------------------------------------------------- builder v4
# v2 pass structure with: bias matmuls removed (bn1 via ACT bias; bn2 must
# be zero), pass-D ssel matmuls interleaved one-for-one behind the next
# group's pass-C matmuls (hides D's LDWEIGHTS exposure), persistent
# ping-pong agg PSUM halves (no group serialization), pass A packing two
# blocks per PSUM tile via 4 col-strips, PSUM evictions on ACT/DVE only.

def build_nc_v4(nb=BPC, reps=1, hw_loop=False,
                comb_pat="D", m1_pat="ADAD", m2_pat="AADADAADDA", agg_pat="D",
                drain_c=1, drain_n=3):
    import contextlib
    import concourse.bass as bass
    import concourse.mybir as mybir
    import concourse.tile as tile

    f32, bf16 = mybir.dt.float32, mybir.dt.bfloat16
    AF = mybir.ActivationFunctionType
    ALU = mybir.AluOpType
    nn = nb * NPB
    nts = min(512, nn)
    nt = nn // nts
    ngr = max(1, nb // 4)
    assert nb % 4 == 0 or nb == 2

    nc = bass.Bass()
    h0T_d = nc.dram_tensor("h0T", [2, 128, nn], f32, kind="ExternalInput")
    crg_d = nc.dram_tensor("crg", [nb, 128, EPB], bf16, kind="ExternalInput")
    we2r_d = nc.dram_tensor("we2r", [128, L * 2 * 256], bf16, kind="ExternalInput")
    w1ab_d = nc.dram_tensor("w1ab", [128, L * 2 * 2 * 256], bf16,
                            kind="ExternalInput")
    w1dp_d = nc.dram_tensor("w1dp", [64, L * 256], bf16, kind="ExternalInput")
    wn1a_d = nc.dram_tensor("wn1a", [128, L * 2 * 2 * 128], bf16,
                            kind="ExternalInput")
    wn1b_d = nc.dram_tensor("wn1b", [128, L * 2 * 2 * 128], bf16,
                            kind="ExternalInput")
    wn2_d = nc.dram_tensor("wn2", [128, L * 2 * 2 * 128], bf16,
                           kind="ExternalInput")
    biasv3_d = nc.dram_tensor("biasv3", [128, 5 * L * 2], f32,
                              kind="ExternalInput")
    ssel_d = nc.dram_tensor("ssel", [128, 5 * NPB], bf16, kind="ExternalInput")
    out_d = nc.dram_tensor("poolT", [2, 128, nb], f32, kind="ExternalOutput")

    with tile.TileContext(nc) as tc:
        with (
            tc.tile_pool(name="const", bufs=1) as csp,
            tc.tile_pool(name="crgp", bufs=1) as crgp,
            tc.tile_pool(name="state", bufs=1) as stp,
            tc.tile_pool(name="comb", bufs=1) as combp,
            tc.tile_pool(name="m1p", bufs=5) as m1p,
            tc.tile_pool(name="m2p", bufs=26) as m2p,
            tc.tile_pool(name="ps", bufs=7, space="PSUM") as psp,
            tc.tile_pool(name="psagg", bufs=1, space="PSUM") as psaggp,
        ):
            we2r_s = csp.tile([128, L * 2 * 256], bf16, name="we2r_s")
            nc.sync.dma_start(we2r_s[:], we2r_d[:])
            w1ab_s = csp.tile([128, L * 2 * 2 * 256], bf16, name="w1ab_s")
            nc.sync.dma_start(w1ab_s[:], w1ab_d[:])
            w1dp_s = csp.tile([128, L * 256], bf16, name="w1dp_s")
            nc.sync.dma_start(w1dp_s[64:128, :], w1dp_d[:])
            wn1a_s = csp.tile([128, L * 2 * 2 * 128], bf16, name="wn1a_s")
            nc.sync.dma_start(wn1a_s[:], wn1a_d[:])
            wn1b_s = csp.tile([128, L * 2 * 2 * 128], bf16, name="wn1b_s")
            nc.sync.dma_start(wn1b_s[:], wn1b_d[:])
            wn2_s = csp.tile([128, L * 2 * 2 * 128], bf16, name="wn2_s")
            nc.sync.dma_start(wn2_s[:], wn2_d[:])
            bias_s = csp.tile([128, 5 * L * 2], f32, name="bias_s")
            nc.sync.dma_start(bias_s[:], biasv3_d[:])
            ssel_s = csp.tile([128, 5 * NPB], bf16, name="ssel_s")
            nc.sync.dma_start(ssel_s[:], ssel_d[:])

            def we2r_ap(l, kc):
                o = (l * 2 + kc) * 256
                return we2r_s[:, o:o + 256]

            def w1ab_ap(l, sel, kc):
                o = ((l * 2 + sel) * 2 + kc) * 256
                return w1ab_s[:, o:o + 256]

            def wfam_ap(t, l, kc, mc):
                o = ((l * 2 + kc) * 2 + mc) * 128
                return t[:, o:o + 128]

            def bv3(j, l, mc):
                o = (j * L + l) * 2 + mc
                return bias_s[:, o:o + 1]

            hT, hbf, aggbf, n1bf = [], [], [], []
            for c in range(2):
                hT.append(stp.tile([128, nn], f32, name=f"hT{c}", tag=f"hT{c}"))
                hbf.append(stp.tile([128, nn], bf16, name=f"hbf{c}", tag=f"hbf{c}"))
                aggbf.append(stp.tile([128, nn], bf16, name=f"agb{c}", tag=f"agb{c}"))
                n1bf.append(stp.tile([128, nn], bf16, name=f"n1b{c}", tag=f"n1b{c}"))

            agg_ps = psaggp.tile([128, 512], f32, name="agg_ps", tag="aggps")

            if not hw_loop:
                for c in range(2):
                    for t in range(nt):
                        sl = slice(t * nts, (t + 1) * nts)
                        nc.sync.dma_start(hT[c][:, sl], h0T_d[c][:, sl])
                        nc.gpsimd.tensor_copy(hbf[c][:, sl], hT[c][:, sl])

            crg_s = []
            for b in range(nb):
                t = crgp.tile([128, EPB], bf16, name=f"crg{b}", tag=f"crg{b}")
                nc.sync.dma_start(t[:], crg_d[b])
                crg_s.append(t)

            comb_tiles = [
                [combp.tile([128, 256], bf16, tag=f"comb{l}_{i}",
                            name=f"comb{l}_{i}") for i in range(min(4, nb))]
                for l in range(L)]

            ev_i = {"comb": 0, "m1": 0, "m2": 0, "agg": 0}
            pats = {"comb": comb_pat, "m1": m1_pat, "m2": m2_pat, "agg": agg_pat}

            def eng(kind):
                ch = pats[kind][ev_i[kind] % len(pats[kind])]
                ev_i[kind] += 1
                return ch

            def ev_copy(kind, out_ap, ps_ap):
                if eng(kind) == "A":
                    nc.scalar.copy(out_ap, ps_ap)
                else:
                    nc.vector.tensor_copy(out_ap, ps_ap)

            def ev_relu(kind, out_ap, ps_ap, jpos, jneg, l, mc):
                if eng(kind) == "A":
                    nc.scalar.activation(out_ap, ps_ap, AF.Relu,
                                         bias=bv3(jpos, l, mc))
                else:
                    nc.vector.scalar_tensor_tensor(
                        out_ap, ps_ap, bv3(jneg, l, mc),
                        bv3(jpos, l, mc).to_broadcast(
                            [out_ap.shape[0], out_ap.free_size()]),
                        op0=ALU.max, op1=ALU.add)

            # D-instruction queue (thunks); drained behind later PE work
            dq = []

            def drain(k):
                for _ in range(min(k, len(dq))):
                    dq.pop(0)()

            loop_ctx = (tc.For_i(0, reps, 1) if hw_loop else contextlib.nullcontext())
            rep_range = range(1 if hw_loop else reps)
            with loop_ctx:
             for rep in rep_range:
              if hw_loop or rep > 0:
                for c in range(2):
                    for t in range(nt):
                        sl = slice(t * nts, (t + 1) * nts)
                        nc.sync.dma_start(hT[c][:, sl], h0T_d[c][:, sl])
                        nc.gpsimd.tensor_copy(hbf[c][:, sl], hT[c][:, sl])
              for l in range(L):
                for i in range(min(4, nb)):
                    nc.gpsimd.tensor_copy(
                        comb_tiles[l][i][64:128, :],
                        w1dp_s[64:128, l * 256:(l + 1) * 256])

                def emit_A(g):
                    # two blocks per PSUM tile, 4 col-strips
                    for half in range(2):
                        b0 = g * 4 + half * 2
                        if b0 >= nb:
                            return
                        ps_ab = psp.tile([128, 512], f32, tag="ps",
                                         name=f"ab{l}_{b0}")
                        for bi in range(2):
                            b = b0 + bi
                            if b >= nb:
                                break
                            for sel in range(2):
                                pos0 = bi * 64 + sel * 32
                                for kc in range(2):
                                    nc.tensor.matmul(
                                        ps_ab[pos0:pos0 + 32, 0:256],
                                        lhsT=hbf[kc][:, b * NPB:(b + 1) * NPB],
                                        rhs=w1ab_ap(l, sel, kc),
                                        start=(kc == 0), stop=(kc == 1),
                                        tile_position=(0, pos0),
                                        skip_group_check=True)
                        for bi in range(2):
                            b = b0 + bi
                            if b >= nb:
                                break
                            comb = comb_tiles[l][b % 4]
                            ev_copy("comb", comb[0:64, :],
                                    ps_ab[bi * 64:bi * 64 + 64, 0:256])

                def emit_B(b):
                    comb = comb_tiles[l][b % 4]
                    m1t = [m1p.tile([128, EPB], bf16, tag=f"m1_{kc}",
                                    name=f"m1_{l}_{b}_{kc}") for kc in range(2)]
                    for kc in range(2):
                        for h in range(2):
                            ps1 = psp.tile([128, 512], f32, tag="ps",
                                           name=f"p1_{l}_{b}_{kc}_{h}")
                            nc.tensor.matmul(
                                ps1[:, 0:HALF],
                                lhsT=comb[:, kc * 128:(kc + 1) * 128],
                                rhs=crg_s[b][:, h * HALF:(h + 1) * HALF],
                                start=True, stop=True, skip_group_check=True)
                            ev_relu("m1", m1t[kc][:, h * HALF:(h + 1) * HALF],
                                    ps1[:, 0:HALF], 0, 1, l, kc)
                    return m1t

                def emit_C(b, m1t):
                    m2sbs = []
                    for p in range(3):
                        ecs = (2 * p, 2 * p + 1) if p < 2 else (4,)
                        w = 256 * len(ecs)
                        ps2 = psp.tile([128, 512], f32, tag="ps",
                                       name=f"p2_{l}_{b}_{p}")
                        for j, ec in enumerate(ecs):
                            for kc in range(2):
                                nc.tensor.matmul(
                                    ps2[:, j * 256:(j + 1) * 256],
                                    lhsT=m1t[kc][:, ec * 128:(ec + 1) * 128],
                                    rhs=we2r_ap(l, kc),
                                    start=(kc == 0), stop=(kc == 1),
                                    skip_group_check=True)
                                drain(drain_c)
                        m2sb = m2p.tile([128, 512], bf16, tag="m2",
                                        name=f"m2_{l}_{b}_{p}")
                        for j in range(len(ecs)):
                            jm = j * 256
                            ev_relu("m2", m2sb[:, jm:jm + 256],
                                    ps2[:, jm:jm + 256], 2, 3, l, 0)
                        m2sbs.append(m2sb)
                    return m2sbs

                def queue_D(g, m2ss):
                    half = (g % 2) * 256

                    def mk_mm(bi, mc, ec):
                        p, j = divmod(ec, 2)
                        m2sb = m2ss[bi]

                        def f():
                            nc.tensor.matmul(
                                agg_ps[:, half + mc * 128 + bi * 32:
                                       half + mc * 128 + bi * 32 + 32],
                                lhsT=m2sb[p][:, j * 256 + mc * 128:
                                             j * 256 + (mc + 1) * 128],
                                rhs=ssel_s[:, ec * NPB:(ec + 1) * NPB],
                                start=(ec == 0), stop=(ec == 4),
                                skip_group_check=True)
                        return f

                    for bi in range(min(4, nb)):
                        for mc in range(2):
                            for ec in range(5):
                                dq.append(mk_mm(bi, mc, ec))

                    def mk_ev(mc):
                        def f():
                            ev_copy("agg", aggbf[mc][:, g * 128:(g + 1) * 128],
                                    agg_ps[:, half + mc * 128:
                                           half + (mc + 1) * 128])
                        return f
                    for mc in range(2):
                        dq.append(mk_ev(mc))

                # ---------------- edge phase, software-pipelined groups
                emit_A(0)
                for g in range(ngr):
                    m1ts = {}
                    for bi in range(min(4, nb)):
                        m1ts[bi] = emit_B(g * 4 + bi)
                    if g + 1 < ngr:
                        emit_A(g + 1)
                    m2ss = {}
                    for bi in range(min(4, nb)):
                        m2ss[bi] = emit_C(g * 4 + bi, m1ts[bi])
                        drain(2)
                    queue_D(g, m2ss)
                # ---------------- node phase (drains the last D group)
                for t in range(nt):
                    if t == nt - 1:
                        drain(len(dq))
                    for mc in range(2):
                        sl = slice(t * nts, (t + 1) * nts)
                        psn = psp.tile([128, 512], f32, tag="ps",
                                       name=f"n1_{l}_{mc}_{t}")
                        for kc in range(2):
                            nc.tensor.matmul(psn[:, 0:nts],
                                             lhsT=wfam_ap(wn1a_s, l, kc, mc),
                                             rhs=hbf[kc][:, sl],
                                             start=(kc == 0), stop=False,
                                             skip_group_check=True)
                            drain(drain_n)
                            nc.tensor.matmul(psn[:, 0:nts],
                                             lhsT=wfam_ap(wn1b_s, l, kc, mc),
                                             rhs=aggbf[kc][:, sl],
                                             start=False, stop=(kc == 1),
                                             skip_group_check=True)
                            drain(drain_n)
                        nc.scalar.activation(n1bf[mc][:, sl], psn[:, 0:nts],
                                             AF.Relu, bias=bv3(4, l, mc))
                drain(len(dq))
                for t in range(nt):
                    for mc in range(2):
                        sl = slice(t * nts, (t + 1) * nts)
                        pso = psp.tile([128, 512], f32, tag="ps",
                                       name=f"n2_{l}_{mc}_{t}")
                        for kc in range(2):
                            nc.tensor.matmul(pso[:, 0:nts],
                                             lhsT=wfam_ap(wn2_s, l, kc, mc),
                                             rhs=n1bf[kc][:, sl],
                                             start=(kc == 0), stop=(kc == 1),
                                             skip_group_check=True)
                        nc.vector.scalar_tensor_tensor(
                            hT[mc][:, sl], hT[mc][:, sl], 2.0, pso[:, 0:nts],
                            op0=ALU.mult, op1=ALU.add)
                        if l + 1 < L:
                            nc.gpsimd.tensor_copy(hbf[mc][:, sl], hT[mc][:, sl])
              # ---------------- pooling
              for mc in range(2):
                pool_t = stp.tile([128, nb], f32, tag=f"pool{mc}", name=f"pool{mc}")
                nc.vector.tensor_reduce(pool_t[:],
                                        hT[mc][:].rearrange("p (n k) -> p n k", k=NPB),
                                        axis=mybir.AxisListType.X, op=ALU.add)
                nc.scalar.mul(pool_t[:], pool_t[:], 1.0 / NPB)
                nc.sync.dma_start(out_d[mc], pool_t[:])
    return nc


# ---------------------------------------------- numpy model of the v3 math

def numpy_model_v3(ins, extras, nb=BPC, cores=None, dr_layers=DR_LAYERS):
    E4 = ml_dtypes.float8_e4m3fn

    def b16(x):
        return x.astype(BF16).astype(np.float32)

    def f8(x):
        return x.astype(E4).astype(np.float32)

    we2dr = np.asarray(extras["we2dr"], np.float32).reshape(128, L, 2, 2, 128)
    w1dpv3 = np.asarray(extras["w1dpv3"], np.float32).reshape(64, L, 256)
    bias = np.asarray(extras["biasv3"], np.float32)

    def bcol(j, l):
        return np.concatenate([bias[:, (j * L + l) * 2 + mc] for mc in range(2)])

    outs = []
    for m in (ins if cores is None else [ins[c] for c in cores]):
        h = np.asarray(m["h0T"], np.float32).reshape(256, -1)[:, :nb * NPB]
        crg = np.asarray(m["crg"], np.float32)[:nb]
        w1ab = np.asarray(m["w1ab"], np.float32).reshape(
            128, L, 2, 2, 256).transpose(1, 2, 3, 0, 4)
        we2 = np.asarray(m["we2"], np.float32).reshape(
            128, L, 2, 2, 128).transpose(1, 2, 3, 0, 4)
        wn1a = np.asarray(m["wn1a"], np.float32).reshape(
            128, L, 2, 2, 128).transpose(1, 2, 3, 0, 4)
        wn1b = np.asarray(m["wn1b"], np.float32).reshape(
            128, L, 2, 2, 128).transpose(1, 2, 3, 0, 4)
        wn2 = np.asarray(m["wn2"], np.float32).reshape(
            128, L, 2, 2, 128).transpose(1, 2, 3, 0, 4)
        nn = nb * NPB

        def blk(w):
            return np.concatenate(
                [np.concatenate([w[kc_, mc_] for mc_ in range(2)], axis=1)
                 for kc_ in range(2)], axis=0)

        for l in range(L):
            is_dr = l in dr_layers
            cm = _cm(l, dr_layers)
            hb = b16(h)
            W1b = np.concatenate([w1ab[l, 0, kc_] for kc_ in range(2)], axis=0)
            W1a = np.concatenate([w1ab[l, 1, kc_] for kc_ in range(2)], axis=0)
            if is_dr:
                # We2 pack slice (l, mc): [128, kc, m]; logical [256, 256]
                W2 = np.concatenate(
                    [np.concatenate([we2dr[:, l, mc, kc, :] for mc in range(2)],
                                    axis=1) for kc in range(2)], axis=0)
                s2 = 1.0 / (cm * CW)
            else:
                W2 = blk(we2[l])
                s2 = 1.0
            be1c = bcol(0, l)
            be2 = bcol(2, l)
            bn1 = bcol(4, l)
            agg = np.zeros((256, nn), np.float32)
            for b in range(nb):
                hs = hb[:, b * NPB:(b + 1) * NPB]
                hB = b16(hs.T @ W1b * cm)
                hA = b16(hs.T @ W1a * cm)
                combined = np.concatenate([hB, hA, w1dpv3[:, l, :]], axis=0)
                pre1 = combined.T @ crg[b]
                m1 = np.maximum(pre1 + be1c[:, None], 0.0)
                m1 = f8(m1) if is_dr else b16(m1)
                m2 = b16(np.maximum(W2.T @ m1 * s2 + be2[:, None], 0.0))
                agg[:, b * NPB:(b + 1) * NPB] = b16(
                    m2.reshape(256, NPB, K).sum(axis=2))
            aggb = agg
            N1a, N1b_, N2 = blk(wn1a[l]), blk(wn1b[l]), blk(wn2[l])
            n1 = b16(np.maximum(N1a.T @ hb + N1b_.T @ aggb + bn1[:, None], 0.0))
            out = N2.T @ n1
            h = 2.0 * h + out
        pooled = h.reshape(256, nb, NPB).mean(axis=2)
        outs.append(pooled.T)
    return np.concatenate(outs, axis=0)


# ===================================================================== entry

_CACHE = {}


def _get_runner(be2_mm):
    key = ("runner", be2_mm)
    if key not in _CACHE:
        apply_tilefix()
        nc = build_nc_v2(nb=BPC, be2_mm=be2_mm,
                         m1_dve_of_8=4, m2_dve_of_8=4,
                         comb_dve_of_8=5, agg_dve_of_8=2)
        split_waits(nc, cap=1, cap_sp=1)
        _CACHE[key] = nc
    return _CACHE[key]


def _get_runner_v4():
    key = "runner_v4"
    if key not in _CACHE:
        apply_tilefix()
        nc = build_nc_v4(nb=BPC)
        split_waits(nc, cap=1, cap_sp=1)
        _CACHE[key] = nc
    return _CACHE[key]


def _run(nc, per_core):
    import concourse.mybir as mybir
    from concourse.bass_utils import run_bass_kernel_spmd
    declared = set()
    for alloc in nc.m.functions[0].allocations:
        if isinstance(alloc, mybir.MemoryLocationSet) and alloc.kind == "ExternalInput":
            declared.add(alloc.memorylocations[0].name)
    in_maps = [{k: v for k, v in m.items() if k in declared} for m in per_core]
    res = run_bass_kernel_spmd(nc, in_maps, core_ids=list(range(NCORES)))
    return host_unshard(res.results).astype(np.float32)


def kernel(**inputs):
    """Full inputs in (as in reference.setup_inputs), full [B, 256] f32 out."""
    np_inputs = {k: np.asarray(v) for k, v in inputs.items()}
    per_core = host_prep(**np_inputs)
    if np.any(np.asarray(np_inputs["bn2"]) != 0):
        # v4 folds bn2 away only when it is zero; exact fallback to v2
        be2_mm = bool(per_core[0]["be2_nonzero"][0])
        return _run(_get_runner(be2_mm), per_core)
    extras = host_prep_v3(np_inputs)
    per_core = [{**m, **extras} for m in per_core]
    return _run(_get_runner_v4(), per_core)



# revision 24
# speedup vs baseline: 1.0998x; 1.0998x over previous
"""BuildingBlockEmbedder GNN kernel for trn2 — shared library.

Layout: feature-on-partition ("transposed") everywhere on device.
Per core: 64 building blocks x 32 atoms = 2048 nodes, 40960 edges.
"""
import numpy as np
import ml_dtypes

BF16 = ml_dtypes.bfloat16

# problem constants
NUM_GAUSS = 64
MAX_R = 5.0
L = 4
C = 256            # node/hidden channels
NPB = 32           # atoms per block
K = 20             # neighbors
B = 512            # blocks
N = B * NPB
E = N * K
GAUSS_COEFF = -0.5 / (MAX_R / (NUM_GAUSS - 1)) ** 2
NCORES = 8
BPC = B // NCORES          # 64 blocks per core
NPC = BPC * NPB            # 2048 nodes per core
EPB = NPB * K              # 640 edges per block
HALF = EPB // 2            # 320-edge matmul unit

# ---------------------------------------------------------------- host prep

def host_prep(local_coords, atom_types, edge_index, batch_bb, atom_embed, offset,
              We1, be1, We2, be2, Wn1, bn1, Wn2, bn2):
    """Build per-core device inputs from full problem inputs (all numpy)."""
    pos = np.asarray(local_coords, np.float32)
    types = np.asarray(atom_types).astype(np.int64)
    ei = np.asarray(edge_index).astype(np.int64)
    We1 = np.asarray(We1, np.float32); be1 = np.asarray(be1, np.float32)
    We2 = np.asarray(We2, np.float32); be2 = np.asarray(be2, np.float32)
    Wn1 = np.asarray(Wn1, np.float32); bn1 = np.asarray(bn1, np.float32)
    Wn2 = np.asarray(Wn2, np.float32); bn2 = np.asarray(bn2, np.float32)
    emb = np.asarray(atom_embed, np.float32)

    row, col = ei[0], ei[1]
    # structural assumptions from the reference graph builder
    assert np.array_equal(row, np.repeat(np.arange(N, dtype=np.int64), K)), \
        "edge rows must be repeat(arange(N), K)"
    assert np.all(col // NPB == row // NPB), "edges must stay within blocks"

    dvec = pos[col] - pos[row]
    d = np.sqrt((dvec * dvec).sum(-1))          # [E] Angstrom
    assert d.max() < MAX_R - 0.55, f"d.max()={d.max()}: last gaussian not negligible"
    radial = (0.01 * d * d).astype(np.float32)  # ANG_TO_NM^2 * d^2
    # gaussians 0..62 (63rd is exp(<-30) ~= 0 for all d here; its row carries radial)
    off = np.asarray(offset, np.float32)
    gauss = np.exp(GAUSS_COEFF * (d[:, None] - off[None, :63]) ** 2).astype(np.float32)

    # CRG [B, 128, EPB]: rows 0-31 C_sel, 32-63 R_sel, 64 radial, 65-127 gauss
    col_local = (col - (row // NPB) * NPB).astype(np.int32).reshape(B, EPB)
    crg = np.zeros((B, 128, EPB), np.float32)
    e_ar = np.arange(EPB)
    r_sel = np.zeros((NPB, EPB), np.float32)
    r_sel[e_ar // K, e_ar] = 1.0
    for b in range(B):
        crg[b, col_local[b], e_ar] = 1.0      # C_sel
    crg[:, 32:64, :] = r_sel[None]
    crg[:, 64, :] = radial.reshape(B, EPB)
    crg[:, 65:, :] = gauss.reshape(B, EPB, 63).transpose(0, 2, 1)
    crg = crg.astype(BF16)

    h0 = emb[types - 1]                        # [N, C] f32
    h0T = h0.reshape(NCORES, NPC, C).transpose(0, 2, 1).reshape(
        NCORES, 2, 128, NPC).copy()            # [core, chunk, 128, 2048]

    def chunks_lhsT(w):   # w [L, 256, 256] -> [L, kc, mc, 128, 128] bf16
        return np.ascontiguousarray(
            w.reshape(L, 2, 128, 2, 128).transpose(0, 1, 3, 2, 4)).astype(BF16)

    w1b = np.ascontiguousarray(
        We1[:, 256:512, :].reshape(L, 2, 128, 256))      # rhs [L, kc, 128, 256]
    w1a = np.ascontiguousarray(We1[:, 0:256, :].reshape(L, 2, 128, 256))
    w1ab = np.stack([w1b, w1a], axis=1).astype(BF16)     # [L, 2(b,a), kc, 128, 256]
    w1dp = np.concatenate([We1[:, 512:513, :], We1[:, 513:576, :]],
                          axis=1).astype(BF16)           # [L, 64, 256]
    we2 = chunks_lhsT(We2)
    wn1a = chunks_lhsT(Wn1[:, 0:256, :])
    wn1b = chunks_lhsT(Wn1[:, 256:512, :])
    wn2 = chunks_lhsT(Wn2)
    # bias K=1 lhsT rows [L, 2(bn1,bn2), mc, 1, 128] bf16
    bnrow = np.stack([bn1, bn2], axis=1).reshape(L, 2, 2, 1, 128).astype(BF16)
    # ACT/DVE bias columns [128, 4*L*2]; col = (j*L + l)*2 + mc
    # j: 0 = be1, 1 = be2, 2 = -be2, 3 = -be1
    be12 = np.zeros((128, 4 * L * 2), np.float32)
    for j, bb in enumerate([be1, be2, -be2, -be1]):
        for l in range(L):
            for mc in range(2):
                be12[:, (j * L + l) * 2 + mc] = bb[l, mc * 128:(mc + 1) * 128]

    def pmaj(w):  # [L, kc, mc, 128p, 128q] -> [128, L*kc*mc*128]
        return np.ascontiguousarray(w.transpose(3, 0, 1, 2, 4).reshape(128, -1))

    # v2 extras: We2 as rhs [128, L*2*256]; be2 repeated row; Ssel matrices
    we2r = np.ascontiguousarray(
        We2.reshape(L, 2, 128, 256).transpose(2, 0, 1, 3).reshape(128, -1)).astype(BF16)
    be2row = np.ascontiguousarray(
        np.repeat(be2[:, None, :], 2, axis=1).reshape(1, L * 512)).astype(BF16)
    ssel = np.zeros((128, 5 * NPB), np.float32)
    for ec in range(5):
        j = np.arange(128)
        ssel[j, ec * NPB + (ec * 128 + j) // K] = 1.0
    ssel = np.ascontiguousarray(ssel).astype(BF16)

    shared = dict(
        we2r=we2r, be2row=be2row, ssel=ssel,
        be2_nonzero=np.asarray([1.0 if np.any(be2 != 0) else 0.0], np.float32),
        we2=pmaj(we2),
        w1ab=np.ascontiguousarray(w1ab.transpose(3, 0, 1, 2, 4).reshape(128, -1)),
        w1dp=np.ascontiguousarray(w1dp.transpose(1, 0, 2).reshape(64, -1)),
        wn1a=pmaj(wn1a), wn1b=pmaj(wn1b), wn2=pmaj(wn2),
        bnrow=np.ascontiguousarray(bnrow.transpose(3, 0, 1, 2, 4).reshape(1, -1)),
        be12=be12)
    per_core = []
    for c in range(NCORES):
        m = dict(h0T=h0T[c], crg=crg[c * BPC:(c + 1) * BPC])
        m.update(shared)
        per_core.append(m)
    return per_core


def host_unshard(results):
    """results: list of 8 dicts with 'poolT' [2,128,nb] -> full [B, 256] f32."""
    outs = []
    for r in results:
        pt = np.asarray(r["poolT"], np.float32)      # [2, 128, nb]
        nb = pt.shape[2]
        outs.append(pt.reshape(256, nb).T)           # [nb, 256]
    return np.concatenate(outs, axis=0)


# ------------------------------------------------------------ tile drain fix

def apply_tilefix():
    """This container's walrus allows only ONE sem-wait on an SP Drain —
    split the Tile tail-drain waits across serial drains."""
    import concourse.mybir as mybir
    import concourse.tile as tile
    from concourse.tile import ScopedClock

    if getattr(tile.TileContext, "_drain_fix_applied", False):
        return

    def _split(self, tick_clock, wait_clock):
        d = self.nc.sync.drain()
        wait_clock.add_sem_waits(d.ins, ScopedClock({None: tick_clock.global_clock}))
        ws = list(d.ins.sync_info.on_wait) if d.ins.sync_info is not None else []
        if len(ws) > 1:
            d.ins.sync_info.on_wait = ws[:1]
            for w in ws[1:]:
                e = self.nc.sync.drain()
                e.ins.sync_info = mybir.SyncInfo(on_update=[], on_wait=[w])
        self.nc.all_engine_barrier()
        assert self.sems is not None
        popped = self.nc._tile_sem_poison_stack.pop()
        assert popped is self._sem_poison
        self.nc.clear_and_free_semaphores(list(self.sems.allocated().values()))
        self.nc.all_engine_barrier()

    tile.TileContext._drain_and_barrier = _split
    tile.TileContext._drain_fix_applied = True


# ---------------------------------------------------- wait-splitting post-pass

def split_waits(nc, cap=1, cap_sp=1):
    """walrus in this container caps sem-waits per instruction. Hoist excess
    waits onto same-engine NOPs emitted just before the instruction."""
    import concourse.mybir as mybir
    k = 0
    for fn in nc.m.functions:
        for bb in fn.blocks:
            out = []
            for inst in bb.instructions:
                si = inst.sync_info
                ws = list(si.on_wait) if si is not None else []
                c = cap_sp if inst.engine == mybir.EngineType.SP else cap
                if len(ws) > c:
                    keep = ws[:c] if c > 0 else []
                    rest = ws[c:] if c > 0 else ws
                    while rest:
                        chunk, rest = rest[:max(c, 1)], rest[max(c, 1):]
                        nop = mybir.InstNoOp(
                            name=f"wsplit-{k}", engine=inst.engine,
                            sync_info=mybir.SyncInfo(on_wait=chunk, on_update=[]),
                            bass_nofuse=True)
                        k += 1
                        out.append(nop)
                    inst.sync_info.on_wait = keep
                out.append(inst)
            bb.instructions[:] = out
    return k


# ------------------------------------------------------------- bass builder

def build_nc(nb=BPC, reps=1, hw_loop=False):
    """Build the per-core Bass module. nb = blocks per core (small for sim)."""
    import concourse.bass as bass
    import concourse.mybir as mybir
    import concourse.tile as tile

    f32, bf16 = mybir.dt.float32, mybir.dt.bfloat16
    AF = mybir.ActivationFunctionType
    ALU = mybir.AluOpType
    nn = nb * NPB                     # nodes this build
    nts = min(512, nn)                # node tile size
    nt = nn // nts                    # node tiles

    nc = bass.Bass()
    h0T_d = nc.dram_tensor("h0T", [2, 128, nn], f32, kind="ExternalInput")
    crg_d = nc.dram_tensor("crg", [nb, 128, EPB], bf16, kind="ExternalInput")
    we2_d = nc.dram_tensor("we2", [128, L * 2 * 2 * 128], bf16, kind="ExternalInput")
    w1ab_d = nc.dram_tensor("w1ab", [128, L * 2 * 2 * 256], bf16, kind="ExternalInput")
    w1dp_d = nc.dram_tensor("w1dp", [64, L * 256], bf16, kind="ExternalInput")
    wn1a_d = nc.dram_tensor("wn1a", [128, L * 2 * 2 * 128], bf16, kind="ExternalInput")
    wn1b_d = nc.dram_tensor("wn1b", [128, L * 2 * 2 * 128], bf16, kind="ExternalInput")
    wn2_d = nc.dram_tensor("wn2", [128, L * 2 * 2 * 128], bf16, kind="ExternalInput")
    bnrow_d = nc.dram_tensor("bnrow", [1, L * 2 * 2 * 128], bf16, kind="ExternalInput")
    be12_d = nc.dram_tensor("be12", [128, 4 * L * 2], f32, kind="ExternalInput")
    out_d = nc.dram_tensor("poolT", [2, 128, nb], f32, kind="ExternalOutput")

    with tile.TileContext(nc) as tc:
        with (
            tc.tile_pool(name="const", bufs=1) as csp,
            tc.tile_pool(name="crgp", bufs=1) as crgp,
            tc.tile_pool(name="state", bufs=1) as stp,
            tc.tile_pool(name="comb", bufs=4) as combp,
            tc.tile_pool(name="m1p", bufs=3) as m1p,
            tc.tile_pool(name="m2p", bufs=6) as m2p,
            tc.tile_pool(name="ps", bufs=8, space="PSUM") as psp,
        ):
            # ---- resident constants
            we2_s = csp.tile([128, L * 2 * 2 * 128], bf16, name="we2_s")
            nc.sync.dma_start(we2_s[:], we2_d[:])
            w1ab_s = csp.tile([128, L * 2 * 2 * 256], bf16, name="w1ab_s")
            nc.sync.dma_start(w1ab_s[:], w1ab_d[:])
            w1dp_s = csp.tile([128, L * 256], bf16, name="w1dp_s")
            nc.sync.dma_start(w1dp_s[64:128, :], w1dp_d[:])
            wn1a_s = csp.tile([128, L * 2 * 2 * 128], bf16, name="wn1a_s")
            nc.sync.dma_start(wn1a_s[:], wn1a_d[:])
            wn1b_s = csp.tile([128, L * 2 * 2 * 128], bf16, name="wn1b_s")
            nc.sync.dma_start(wn1b_s[:], wn1b_d[:])
            wn2_s = csp.tile([128, L * 2 * 2 * 128], bf16, name="wn2_s")
            nc.sync.dma_start(wn2_s[:], wn2_d[:])
            bnrow_s = csp.tile([128, L * 2 * 2 * 128], bf16, name="bnrow_s")
            nc.sync.dma_start(bnrow_s[0:1, :], bnrow_d[:])
            be12_s = csp.tile([128, 4 * L * 2], f32, name="be12_s")
            nc.sync.dma_start(be12_s[:], be12_d[:])
            ones_s = csp.tile([128, 512], bf16, name="ones_s")
            nc.gpsimd.memset(ones_s[0:1, :], 1.0)

            def we2_ap(l, kc, mc):
                o = ((l * 2 + kc) * 2 + mc) * 128
                return we2_s[:, o:o + 128]

            def w1ab_ap(l, s, kc):
                o = ((l * 2 + s) * 2 + kc) * 256
                return w1ab_s[:, o:o + 256]

            def wfam_ap(t, l, kc, mc):
                o = ((l * 2 + kc) * 2 + mc) * 128
                return t[:, o:o + 128]

            def bnrow_ap(l, j, mc):
                o = ((l * 2 + j) * 2 + mc) * 128
                return bnrow_s[0:1, o:o + 128]

            def be_ap(j, l, mc):
                o = (j * L + l) * 2 + mc
                return be12_s[:, o:o + 1]

            # ---- CRG resident
            crg_s = []
            for b in range(nb):
                t = crgp.tile([128, EPB], bf16, name=f"crg{b}", tag=f"crg{b}")
                nc.sync.dma_start(t[:], crg_d[b])
                crg_s.append(t)

            # ---- state
            hT, hbf, aggT, aggbf, n1bf = [], [], [], [], []
            for c in range(2):
                t = stp.tile([128, nn], f32, name=f"hT{c}", tag=f"hT{c}")
                hT.append(t)
                hbf.append(stp.tile([128, nn], bf16, name=f"hbf{c}", tag=f"hbf{c}"))
                aggT.append(stp.tile([128, nn], f32, name=f"aggT{c}", tag=f"aggT{c}"))
                aggbf.append(stp.tile([128, nn], bf16, name=f"aggbf{c}", tag=f"agb{c}"))
                n1bf.append(stp.tile([128, nn], bf16, name=f"n1bf{c}", tag=f"n1b{c}"))

            import contextlib
            loop_ctx = (tc.For_i(0, reps, 1) if hw_loop
                        else contextlib.nullcontext())
            rep_range = range(1 if hw_loop else reps)
            with loop_ctx:
             for rep in rep_range:
              for c in range(2):
                nc.sync.dma_start(hT[c][:], h0T_d[c])
              for l in range(L):
                for c in range(2):
                    nc.gpsimd.tensor_copy(hbf[c][:], hT[c][:])
                # ---------------- edge phase, per block
                for b in range(nb):
                    ps_ab = psp.tile([128, 512], f32, tag="ps", name=f"ab{l}_{b}")
                    for sel, pos0 in ((0, 0), (1, 32)):
                        for kc in range(2):
                            nc.tensor.matmul(ps_ab[pos0:pos0 + 32, 0:256],
                                             lhsT=hbf[kc][:, b * NPB:(b + 1) * NPB],
                                             rhs=w1ab_ap(l, sel, kc),
                                             start=(kc == 0), stop=(kc == 1),
                                             tile_position=(0, pos0))
                    comb = combp.tile([128, 256], bf16, tag="comb", name=f"cb{l}_{b}")
                    nc.scalar.copy(comb[0:64, :], ps_ab[0:64, 0:256])
                    nc.gpsimd.tensor_copy(comb[64:128, :],
                                          w1dp_s[64:128, l * 256:(l + 1) * 256])
                    m1t = [m1p.tile([128, EPB], bf16, tag=f"m1_{kc}",
                                    name=f"m1_{l}_{b}_{kc}") for kc in range(2)]
                    for mc in range(2):
                        for h in range(2):
                            ps1 = psp.tile([128, 512], f32, tag="ps",
                                           name=f"p1_{l}_{b}_{mc}_{h}")
                            nc.tensor.matmul(ps1[:, 0:HALF],
                                             lhsT=comb[:, mc * 128:(mc + 1) * 128],
                                             rhs=crg_s[b][:, h * HALF:(h + 1) * HALF],
                                             start=True, stop=True)
                            nc.scalar.activation(m1t[mc][:, h * HALF:(h + 1) * HALF],
                                                 ps1[:, 0:HALF], AF.Relu,
                                                 bias=be_ap(0, l, mc))
                    for mc in range(2):
                        for h in range(2):
                            ps2 = psp.tile([128, 512], f32, tag="ps",
                                           name=f"p2_{l}_{b}_{mc}_{h}")
                            for kc in range(2):
                                nc.tensor.matmul(
                                    ps2[:, 0:HALF],
                                    lhsT=we2_ap(l, kc, mc),
                                    rhs=m1t[kc][:, h * HALF:(h + 1) * HALF],
                                    start=(kc == 0), stop=(kc == 1))
                            m2t = m2p.tile([128, HALF], bf16, tag="m2",
                                           name=f"m2_{l}_{b}_{mc}_{h}")
                            if (b * 4 + mc * 2 + h) % 2:
                                nc.scalar.activation(m2t[:], ps2[:, 0:HALF], AF.Relu,
                                                     bias=be_ap(1, l, mc))
                            else:
                                # relu(x + b) == max(x, -b) + b
                                nc.vector.scalar_tensor_tensor(
                                    m2t[:], ps2[:, 0:HALF], be_ap(2, l, mc),
                                    be_ap(1, l, mc).to_broadcast([128, HALF]),
                                    op0=ALU.max, op1=ALU.add)
                            nc.vector.tensor_reduce(
                                aggT[mc][:, b * NPB + h * 16: b * NPB + (h + 1) * 16],
                                m2t[:].rearrange("p (n k) -> p n k", k=K),
                                axis=mybir.AxisListType.X, op=ALU.add)
                # ---------------- node phase
                for c in range(2):
                    nc.gpsimd.tensor_copy(aggbf[c][:], aggT[c][:])
                for mc in range(2):
                    for t in range(nt):
                        sl = slice(t * nts, (t + 1) * nts)
                        psn = psp.tile([128, 512], f32, tag="ps",
                                       name=f"n1_{l}_{mc}_{t}")
                        nc.tensor.matmul(psn[:, 0:nts], lhsT=bnrow_ap(l, 0, mc),
                                         rhs=ones_s[0:1, 0:nts], start=True, stop=False)
                        for kc in range(2):
                            nc.tensor.matmul(psn[:, 0:nts],
                                             lhsT=wfam_ap(wn1a_s, l, kc, mc),
                                             rhs=hbf[kc][:, sl], start=False, stop=False)
                            nc.tensor.matmul(psn[:, 0:nts],
                                             lhsT=wfam_ap(wn1b_s, l, kc, mc),
                                             rhs=aggbf[kc][:, sl], start=False,
                                             stop=(kc == 1))
                        nc.scalar.activation(n1bf[mc][:, sl], psn[:, 0:nts], AF.Relu)
                for mc in range(2):
                    for t in range(nt):
                        sl = slice(t * nts, (t + 1) * nts)
                        pso = psp.tile([128, 512], f32, tag="ps",
                                       name=f"n2_{l}_{mc}_{t}")
                        nc.tensor.matmul(pso[:, 0:nts], lhsT=bnrow_ap(l, 1, mc),
                                         rhs=ones_s[0:1, 0:nts], start=True, stop=False)
                        for kc in range(2):
                            nc.tensor.matmul(pso[:, 0:nts],
                                             lhsT=wfam_ap(wn2_s, l, kc, mc),
                                             rhs=n1bf[kc][:, sl], start=False,
                                             stop=(kc == 1))
                        nc.vector.scalar_tensor_tensor(
                            hT[mc][:, sl], hT[mc][:, sl], 2.0, pso[:, 0:nts],
                            op0=ALU.mult, op1=ALU.add)
              # ---------------- pooling
              for mc in range(2):
                pool_t = stp.tile([128, nb], f32, tag=f"pool{mc}", name=f"pool{mc}")
                nc.vector.tensor_reduce(pool_t[:],
                                        hT[mc][:].rearrange("p (n k) -> p n k", k=NPB),
                                        axis=mybir.AxisListType.X, op=ALU.add)
                nc.scalar.mul(pool_t[:], pool_t[:], 1.0 / NPB)
                nc.sync.dma_start(out_d[mc], pool_t[:])
    return nc


# --------------------------------------------------- numpy model of the math

def numpy_model(ins, nb=BPC, cores=None):
    """Replicate the device math (incl. bf16 rounding) for validation.
    ins: list of per-core input dicts (from host_prep). Returns [sum_nb*NCORES? , 256]."""
    outs = []
    for m in (ins if cores is None else [ins[c] for c in cores]):
        h = np.asarray(m["h0T"], np.float32).reshape(256, -1)[:, :nb * NPB]  # [256, nn]
        crg = np.asarray(m["crg"], np.float32)[:nb]
        L4 = L
        we2 = np.asarray(m["we2"], np.float32).reshape(128, L4, 2, 2, 128).transpose(1, 2, 3, 0, 4)
        w1ab = np.asarray(m["w1ab"], np.float32).reshape(128, L4, 2, 2, 256).transpose(1, 2, 3, 0, 4)
        w1dp = np.asarray(m["w1dp"], np.float32).reshape(64, L4, 256).transpose(1, 0, 2)
        wn1a = np.asarray(m["wn1a"], np.float32).reshape(128, L4, 2, 2, 128).transpose(1, 2, 3, 0, 4)
        wn1b = np.asarray(m["wn1b"], np.float32).reshape(128, L4, 2, 2, 128).transpose(1, 2, 3, 0, 4)
        wn2 = np.asarray(m["wn2"], np.float32).reshape(128, L4, 2, 2, 128).transpose(1, 2, 3, 0, 4)
        bnrow = np.asarray(m["bnrow"], np.float32).reshape(1, L4, 2, 2, 128).transpose(1, 2, 3, 0, 4)
        be12 = np.asarray(m["be12"], np.float32)
        nn = nb * NPB

        def b16(x):
            return x.astype(BF16).astype(np.float32)

        def blk(w):  # [kc, mc, 128, 128] -> [256, 256]
            return np.concatenate(
                [np.concatenate([w[kc_, mc_] for mc_ in range(2)], axis=1)
                 for kc_ in range(2)], axis=0)

        for l in range(L):
            hb = b16(h)                                    # [256, nn]
            # hAB per block
            W1b = np.concatenate([w1ab[l, 0, kc_] for kc_ in range(2)], axis=0)
            W1a = np.concatenate([w1ab[l, 1, kc_] for kc_ in range(2)], axis=0)
            be1 = np.concatenate([be12[:, (0 * L + l) * 2 + mc_] for mc_ in range(2)])
            be2 = np.concatenate([be12[:, (1 * L + l) * 2 + mc_] for mc_ in range(2)])
            agg = np.zeros((256, nn), np.float32)
            for b in range(nb):
                hs = hb[:, b * NPB:(b + 1) * NPB]          # [256, 32]
                hB = b16(hs.T @ W1b)                       # [32, 256] evicted bf16
                hA = b16(hs.T @ W1a)
                combined = np.concatenate([hB, hA, w1dp[l]], axis=0)  # [128, 256]
                pre1 = combined.T @ crg[b]                 # [256, EPB]
                m1 = b16(np.maximum(pre1 + be1[:, None], 0.0))
                W2 = blk(we2[l])
                m2 = b16(np.maximum(W2.T @ m1 + be2[:, None], 0.0))
                agg[:, b * NPB:(b + 1) * NPB] = (
                    m2.reshape(256, NPB, K).sum(axis=2))
            aggb = b16(agg)
            N1a, N1b_, N2 = blk(wn1a[l]), blk(wn1b[l]), blk(wn2[l])
            bn1 = bnrow[l, 0].reshape(256)
            bn2 = bnrow[l, 1].reshape(256)
            n1 = b16(np.maximum(N1a.T @ hb + N1b_.T @ aggb + bn1[:, None], 0.0))
            out = N2.T @ n1 + bn2[:, None]
            h = 2.0 * h + out
        pooled = h.reshape(256, nb, NPB).mean(axis=2)       # [256, nb]
        outs.append(pooled.T)
    return np.concatenate(outs, axis=0)


# --------------------------------------------------------------- builder v2
# m2 in normal layout (edges on partitions); segment-sum as PE matmuls with
# constant Ssel matrices; agg evicted straight to bf16.

def build_nc_v2(nb=BPC, reps=1, hw_loop=False, be2_mm=False,
                m1_dve_of_8=2, m2_dve_of_8=5, comb_dve_of_8=0, agg_dve_of_8=0):
    import contextlib
    import concourse.bass as bass
    import concourse.mybir as mybir
    import concourse.tile as tile

    f32, bf16 = mybir.dt.float32, mybir.dt.bfloat16
    AF = mybir.ActivationFunctionType
    ALU = mybir.AluOpType
    nn = nb * NPB
    nts = min(512, nn)
    nt = nn // nts

    nc = bass.Bass()
    h0T_d = nc.dram_tensor("h0T", [2, 128, nn], f32, kind="ExternalInput")
    crg_d = nc.dram_tensor("crg", [nb, 128, EPB], bf16, kind="ExternalInput")
    we2r_d = nc.dram_tensor("we2r", [128, L * 2 * 256], bf16, kind="ExternalInput")
    w1ab_d = nc.dram_tensor("w1ab", [128, L * 2 * 2 * 256], bf16, kind="ExternalInput")
    w1dp_d = nc.dram_tensor("w1dp", [64, L * 256], bf16, kind="ExternalInput")
    wn1a_d = nc.dram_tensor("wn1a", [128, L * 2 * 2 * 128], bf16, kind="ExternalInput")
    wn1b_d = nc.dram_tensor("wn1b", [128, L * 2 * 2 * 128], bf16, kind="ExternalInput")
    wn2_d = nc.dram_tensor("wn2", [128, L * 2 * 2 * 128], bf16, kind="ExternalInput")
    bnrow_d = nc.dram_tensor("bnrow", [1, L * 2 * 2 * 128], bf16, kind="ExternalInput")
    be12_d = nc.dram_tensor("be12", [128, 4 * L * 2], f32, kind="ExternalInput")
    be2row_d = nc.dram_tensor("be2row", [1, L * 512], bf16, kind="ExternalInput")
    ssel_d = nc.dram_tensor("ssel", [128, 5 * NPB], bf16, kind="ExternalInput")
    out_d = nc.dram_tensor("poolT", [2, 128, nb], f32, kind="ExternalOutput")

    with tile.TileContext(nc) as tc:
        with (
            tc.tile_pool(name="const", bufs=1) as csp,
            tc.tile_pool(name="crgp", bufs=1) as crgp,
            tc.tile_pool(name="state", bufs=1) as stp,
            tc.tile_pool(name="comb", bufs=1) as combp,
            tc.tile_pool(name="m1p", bufs=5) as m1p,
            tc.tile_pool(name="m2p", bufs=14) as m2p,
            tc.tile_pool(name="ps", bufs=7, space="PSUM") as psp,
            tc.tile_pool(name="psagg", bufs=1, space="PSUM") as psaggp,
        ):
            we2r_s = csp.tile([128, L * 2 * 256], bf16, name="we2r_s")
            nc.sync.dma_start(we2r_s[:], we2r_d[:])
            w1ab_s = csp.tile([128, L * 2 * 2 * 256], bf16, name="w1ab_s")
            nc.sync.dma_start(w1ab_s[:], w1ab_d[:])
            w1dp_s = csp.tile([128, L * 256], bf16, name="w1dp_s")
            nc.sync.dma_start(w1dp_s[64:128, :], w1dp_d[:])
            wn1a_s = csp.tile([128, L * 2 * 2 * 128], bf16, name="wn1a_s")
            nc.sync.dma_start(wn1a_s[:], wn1a_d[:])
            wn1b_s = csp.tile([128, L * 2 * 2 * 128], bf16, name="wn1b_s")
            nc.sync.dma_start(wn1b_s[:], wn1b_d[:])
            wn2_s = csp.tile([128, L * 2 * 2 * 128], bf16, name="wn2_s")
            nc.sync.dma_start(wn2_s[:], wn2_d[:])
            bnrow_s = csp.tile([128, L * 2 * 2 * 128], bf16, name="bnrow_s")
            nc.sync.dma_start(bnrow_s[0:1, :], bnrow_d[:])
            be12_s = csp.tile([128, 4 * L * 2], f32, name="be12_s")
            nc.sync.dma_start(be12_s[:], be12_d[:])
            be2row_s = csp.tile([128, L * 512], bf16, name="be2row_s")
            nc.sync.dma_start(be2row_s[0:1, :], be2row_d[:])
            ssel_s = csp.tile([128, 5 * NPB], bf16, name="ssel_s")
            nc.sync.dma_start(ssel_s[:], ssel_d[:])
            ones_s = csp.tile([128, 512], bf16, name="ones_s")
            nc.gpsimd.memset(ones_s[0:1, :], 1.0)
            zcol_s = csp.tile([128, 1], f32, name="zcol_s")
            nc.gpsimd.memset(zcol_s[:], 0.0)

            def we2r_ap(l, kc):
                o = (l * 2 + kc) * 256
                return we2r_s[:, o:o + 256]

            def w1ab_ap(l, sel, kc):
                o = ((l * 2 + sel) * 2 + kc) * 256
                return w1ab_s[:, o:o + 256]

            def wfam_ap(t, l, kc, mc):
                o = ((l * 2 + kc) * 2 + mc) * 128
                return t[:, o:o + 128]

            def bnrow_ap(l, j, mc):
                o = ((l * 2 + j) * 2 + mc) * 128
                return bnrow_s[0:1, o:o + 128]

            def be_ap(j, l, mc):
                o = (j * L + l) * 2 + mc
                return be12_s[:, o:o + 1]

            hT, hbf, aggbf, n1bf = [], [], [], []
            for c in range(2):
                hT.append(stp.tile([128, nn], f32, name=f"hT{c}", tag=f"hT{c}"))
                hbf.append(stp.tile([128, nn], bf16, name=f"hbf{c}", tag=f"hbf{c}"))
                aggbf.append(stp.tile([128, nn], bf16, name=f"agb{c}", tag=f"agb{c}"))
                n1bf.append(stp.tile([128, nn], bf16, name=f"n1b{c}", tag=f"n1b{c}"))

            if not hw_loop:
                for c in range(2):
                    for t in range(nt):
                        sl = slice(t * nts, (t + 1) * nts)
                        nc.sync.dma_start(hT[c][:, sl], h0T_d[c][:, sl])
                        nc.gpsimd.tensor_copy(hbf[c][:, sl], hT[c][:, sl])

            crg_s = []
            for b in range(nb):
                t = crgp.tile([128, EPB], bf16, name=f"crg{b}", tag=f"crg{b}")
                nc.sync.dma_start(t[:], crg_d[b])
                crg_s.append(t)


            comb_tiles = [
                [combp.tile([128, 256], bf16, tag=f"comb{l}_{i}",
                            name=f"comb{l}_{i}") for i in range(min(4, nb))]
                for l in range(L)]

            evict_i = [0]

            def evict(out_ap, ps_ap, relu, bias_ap, dve_of_8):
                """PSUM->SBUF eviction on ACT or DVE (round-robin)."""
                use_dve = (evict_i[0] % 8) < dve_of_8
                evict_i[0] += 1
                if relu:
                    if use_dve and bias_ap is None:
                        nc.vector.scalar_tensor_tensor(
                            out_ap, ps_ap, 0.0,
                            zcol_s[:, 0:1].to_broadcast(
                                [out_ap.shape[0], out_ap.free_size()]),
                            op0=ALU.max, op1=ALU.add)
                    elif use_dve:
                        # relu(x + b) == max(x, -b) + b ; bias_ap=(be, neg_be)
                        be, nbe = bias_ap
                        nc.vector.scalar_tensor_tensor(
                            out_ap, ps_ap, nbe,
                            be.to_broadcast([out_ap.shape[0], out_ap.free_size()]),
                            op0=ALU.max, op1=ALU.add)
                    else:
                        nc.scalar.activation(out_ap, ps_ap, AF.Relu,
                                             bias=(bias_ap[0] if bias_ap else 0.0))
                else:
                    if use_dve:
                        nc.vector.tensor_copy(out_ap, ps_ap)
                    else:
                        nc.scalar.copy(out_ap, ps_ap)

            loop_ctx = (tc.For_i(0, reps, 1) if hw_loop else contextlib.nullcontext())
            rep_range = range(1 if hw_loop else reps)
            with loop_ctx:
             for rep in rep_range:
              if hw_loop or rep > 0:
                for c in range(2):
                    for t in range(nt):
                        sl = slice(t * nts, (t + 1) * nts)
                        nc.sync.dma_start(hT[c][:, sl], h0T_d[c][:, sl])
                        nc.gpsimd.tensor_copy(hbf[c][:, sl], hT[c][:, sl])
              for l in range(L):
                for i in range(min(4, nb)):
                    nc.gpsimd.tensor_copy(
                        comb_tiles[l][i][64:128, :],
                        w1dp_s[64:128, l * 256:(l + 1) * 256])
                for g in range(nb // 4):
                    agg_ps = psaggp.tile([128, 256], f32, tag="agg",
                                         name=f"agg{l}_{g}")
                    # ---- pass A: hA/hB for 4 blocks
                    for bi in range(4):
                        b = g * 4 + bi
                        ps_ab = psp.tile([128, 512], f32, tag="ps",
                                         name=f"ab{l}_{b}")
                        for kc in range(2):
                            for sel, pos0 in ((0, 0), (1, 32)):
                                nc.tensor.matmul(
                                    ps_ab[pos0:pos0 + 32, 0:256],
                                    lhsT=hbf[kc][:, b * NPB:(b + 1) * NPB],
                                    rhs=w1ab_ap(l, sel, kc),
                                    start=(kc == 0), stop=(kc == 1),
                                    tile_position=(0, pos0),
                                    skip_group_check=True)
                        comb = comb_tiles[l][b % 4]
                        evict(comb[0:64, :], ps_ab[0:64, 0:256], False, None,
                              comb_dve_of_8)
                    # ---- pass B: edge MLP layer 1 (transposed out)
                    m1ts = {}
                    for bi in range(4):
                        b = g * 4 + bi
                        comb = comb_tiles[l][b % 4]
                        m1t = [m1p.tile([128, EPB], bf16, tag=f"m1_{kc}",
                                        name=f"m1_{l}_{b}_{kc}") for kc in range(2)]
                        m1ts[bi] = m1t
                        for mc in range(2):
                            for h in range(2):
                                ps1 = psp.tile([128, 512], f32, tag="ps",
                                               name=f"p1_{l}_{b}_{mc}_{h}")
                                nc.tensor.matmul(
                                    ps1[:, 0:HALF],
                                    lhsT=comb[:, mc * 128:(mc + 1) * 128],
                                    rhs=crg_s[b][:, h * HALF:(h + 1) * HALF],
                                    start=True, stop=True)
                                evict(m1t[mc][:, h * HALF:(h + 1) * HALF],
                                      ps1[:, 0:HALF], True,
                                      (be_ap(0, l, mc), be_ap(3, l, mc)),
                                      m1_dve_of_8)
                    # ---- pass C: edge MLP layer 2 (normal out)
                    m2ss = {}
                    for bi in range(4):
                        b = g * 4 + bi
                        m1t = m1ts[bi]
                        m2sbs = []
                        for p in range(3):
                            ecs = (2 * p, 2 * p + 1) if p < 2 else (4,)
                            w = 256 * len(ecs)
                            ps2 = psp.tile([128, 512], f32, tag="ps",
                                           name=f"p2_{l}_{b}_{p}")
                            for j, ec in enumerate(ecs):
                                if be2_mm:
                                    nc.tensor.matmul(
                                        ps2[:, j * 256:(j + 1) * 256],
                                        lhsT=ones_s[0:1, 0:128],
                                        rhs=be2row_s[0:1, l * 512:l * 512 + 256],
                                        start=True, stop=False)
                                for kc in range(2):
                                    nc.tensor.matmul(
                                        ps2[:, j * 256:(j + 1) * 256],
                                        lhsT=m1t[kc][:, ec * 128:(ec + 1) * 128],
                                        rhs=we2r_ap(l, kc),
                                        start=(kc == 0 and not be2_mm),
                                        stop=(kc == 1))
                            m2sb = m2p.tile([128, 512], bf16, tag="m2",
                                            name=f"m2_{l}_{b}_{p}")
                            evict(m2sb[:, 0:w], ps2[:, 0:w], True, None,
                                  m2_dve_of_8)
                            m2sbs.append(m2sb)
                        m2ss[bi] = m2sbs
                    # ---- pass D: PE segment-sum into agg psum
                    for bi in range(4):
                        m2sbs = m2ss[bi]
                        for mc in range(2):
                            for ec in range(5):
                                p, j = divmod(ec, 2)
                                nc.tensor.matmul(
                                    agg_ps[:, mc * 128 + bi * 32:
                                           mc * 128 + bi * 32 + 32],
                                    lhsT=m2sbs[p][:, j * 256 + mc * 128:
                                                  j * 256 + (mc + 1) * 128],
                                    rhs=ssel_s[:, ec * NPB:(ec + 1) * NPB],
                                    start=(ec == 0), stop=(ec == 4))
                    # ---- agg eviction for this 4-block group (bf16 cast)
                    for mc in range(2):
                        evict(aggbf[mc][:, g * 128:(g + 1) * 128],
                              agg_ps[:, mc * 128:(mc + 1) * 128], False, None,
                              agg_dve_of_8)
                # ---------------- node phase
                for mc in range(2):
                    for t in range(nt):
                        sl = slice(t * nts, (t + 1) * nts)
                        psn = psp.tile([128, 512], f32, tag="ps",
                                       name=f"n1_{l}_{mc}_{t}")
                        nc.tensor.matmul(psn[:, 0:nts], lhsT=bnrow_ap(l, 0, mc),
                                         rhs=ones_s[0:1, 0:nts],
                                         start=True, stop=False)
                        for kc in range(2):
                            nc.tensor.matmul(psn[:, 0:nts],
                                             lhsT=wfam_ap(wn1a_s, l, kc, mc),
                                             rhs=hbf[kc][:, sl],
                                             start=False, stop=False)
                            nc.tensor.matmul(psn[:, 0:nts],
                                             lhsT=wfam_ap(wn1b_s, l, kc, mc),
                                             rhs=aggbf[kc][:, sl],
                                             start=False, stop=(kc == 1))
                        nc.scalar.activation(n1bf[mc][:, sl], psn[:, 0:nts], AF.Relu)
                for mc in range(2):
                    for t in range(nt):
                        sl = slice(t * nts, (t + 1) * nts)
                        pso = psp.tile([128, 512], f32, tag="ps",
                                       name=f"n2_{l}_{mc}_{t}")
                        nc.tensor.matmul(pso[:, 0:nts], lhsT=bnrow_ap(l, 1, mc),
                                         rhs=ones_s[0:1, 0:nts],
                                         start=True, stop=False)
                        for kc in range(2):
                            nc.tensor.matmul(pso[:, 0:nts],
                                             lhsT=wfam_ap(wn2_s, l, kc, mc),
                                             rhs=n1bf[kc][:, sl],
                                             start=False, stop=(kc == 1))
                        nc.vector.scalar_tensor_tensor(
                            hT[mc][:, sl], hT[mc][:, sl], 2.0, pso[:, 0:nts],
                            op0=ALU.mult, op1=ALU.add)
                        if l + 1 < L:
                            nc.gpsimd.tensor_copy(hbf[mc][:, sl], hT[mc][:, sl])
              # ---------------- pooling
              for mc in range(2):
                pool_t = stp.tile([128, nb], f32, tag=f"pool{mc}", name=f"pool{mc}")
                nc.vector.tensor_reduce(pool_t[:],
                                        hT[mc][:].rearrange("p (n k) -> p n k", k=NPB),
                                        axis=mybir.AxisListType.X, op=ALU.add)
                nc.scalar.mul(pool_t[:], pool_t[:], 1.0 / NPB)
                nc.sync.dma_start(out_d[mc], pool_t[:])
    return nc


# --------------------------------------------------------------- builder v3
# Edge MLP layer 2 as fp8 DoubleRow matmuls (K=256 in one pass, stationary
# We2), m2 produced transposed; segment-sum as DVE grouped reduce (k=20)
# off the PE. Bias matmuls removed (bn1 via ACT bias; bn2 must be zero).
# Per-layer power-of-2 scaling keeps fp8 operands in range:
#   comb/w1dp/be1 scaled by CM[l]; We2 scaled by CW; m2 evict rescales
#   by 1/(CM[l]*CW) via the ACT scale arg (exact, powers of two).

CM_DR = [32.0, 16.0, 4.0, 1.0]   # m1 fp8 scale, used only on DR layers
CW = 8.0
FP8MAX = 240.0
DR_LAYERS = ()                    # layers running edge-MLP2 as fp8 DoubleRow


def _cm(l, dr_layers):
    return CM_DR[l] if l in dr_layers else 1.0


def host_prep_v3(np_inputs, dr_layers=DR_LAYERS):
    """Extra per-core tensors for the v3 builder (on top of host_prep)."""
    We2 = np.asarray(np_inputs["We2"], np.float32)
    We1 = np.asarray(np_inputs["We1"], np.float32)
    be1 = np.asarray(np_inputs["be1"], np.float32)
    be2 = np.asarray(np_inputs["be2"], np.float32)
    bn1 = np.asarray(np_inputs["bn1"], np.float32)
    E4 = ml_dtypes.float8_e4m3fn

    # We2 DoubleRow pack [128, L*2*2*128] fp8; slice (l, mc) = [128, 2(kc), 128]
    w = np.clip(We2 * CW, -FP8MAX, FP8MAX)
    we2dr = np.ascontiguousarray(
        w.reshape(L, 2, 128, 2, 128).transpose(2, 0, 3, 1, 4).reshape(128, -1)
    ).astype(E4)

    # w1dp scaled per layer [64, L*256] bf16
    cm = np.asarray([_cm(l, dr_layers) for l in range(L)], np.float32)
    w1dp = np.concatenate([We1[:, 512:513, :], We1[:, 513:576, :]], axis=1)
    w1dp = w1dp * cm[:, None, None]
    w1dpv3 = np.ascontiguousarray(
        w1dp.transpose(1, 0, 2).reshape(64, -1)).astype(BF16)

    # bias table [128, 5*L*2] f32; col = (j*L + l)*2 + mc
    # j: 0 = be1*cm, 1 = -be1*cm, 2 = be2, 3 = -be2, 4 = bn1
    bias = np.zeros((128, 5 * L * 2), np.float32)
    for l in range(L):
        rows = [be1[l] * cm[l], -be1[l] * cm[l], be2[l], -be2[l], bn1[l]]
        for j, r in enumerate(rows):
            for mc in range(2):
                bias[:, (j * L + l) * 2 + mc] = r[mc * 128:(mc + 1) * 128]

    return dict(we2dr=we2dr, w1dpv3=w1dpv3, biasv3=bias)


def build_nc_v3(nb=BPC, reps=1, hw_loop=False, be2_nz=False,
                dr_layers=DR_LAYERS,
                m1_pat="APDAPDAP", m2_pat="PADPADPA", comb_pat="PPAP",
                red_pat="D"):
    import contextlib
    import concourse.bass as bass
    import concourse.mybir as mybir
    import concourse.tile as tile

    f32, bf16 = mybir.dt.float32, mybir.dt.bfloat16
    fp8 = mybir.dt.float8e4
    AF = mybir.ActivationFunctionType
    ALU = mybir.AluOpType
    DR = mybir.MatmulPerfMode.DoubleRow
    nn = nb * NPB
    nts = min(512, nn)
    nt = nn // nts

    nc = bass.Bass()
    h0T_d = nc.dram_tensor("h0T", [2, 128, nn], f32, kind="ExternalInput")
    crg_d = nc.dram_tensor("crg", [nb, 128, EPB], bf16, kind="ExternalInput")
    we2_d = nc.dram_tensor("we2", [128, L * 2 * 2 * 128], bf16,
                           kind="ExternalInput")
    we2dr_d = nc.dram_tensor("we2dr", [128, L * 2 * 2 * 128], fp8,
                             kind="ExternalInput")
    w1ab_d = nc.dram_tensor("w1ab", [128, L * 2 * 2 * 256], bf16,
                            kind="ExternalInput")
    w1dpv3_d = nc.dram_tensor("w1dpv3", [64, L * 256], bf16, kind="ExternalInput")
    wn1a_d = nc.dram_tensor("wn1a", [128, L * 2 * 2 * 128], bf16,
                            kind="ExternalInput")
    wn1b_d = nc.dram_tensor("wn1b", [128, L * 2 * 2 * 128], bf16,
                            kind="ExternalInput")
    wn2_d = nc.dram_tensor("wn2", [128, L * 2 * 2 * 128], bf16,
                           kind="ExternalInput")
    biasv3_d = nc.dram_tensor("biasv3", [128, 5 * L * 2], f32,
                              kind="ExternalInput")
    out_d = nc.dram_tensor("poolT", [2, 128, nb], f32, kind="ExternalOutput")

    with tile.TileContext(nc) as tc:
        with (
            tc.tile_pool(name="const", bufs=1) as csp,
            tc.tile_pool(name="crgp", bufs=1) as crgp,
            tc.tile_pool(name="state", bufs=1) as stp,
            tc.tile_pool(name="comb", bufs=1) as combp,
            tc.tile_pool(name="m1p", bufs=3) as m1p,
            tc.tile_pool(name="m2p", bufs=8) as m2p,
            tc.tile_pool(name="ps", bufs=1, space="PSUM") as psp,
        ):
            we2_s = csp.tile([128, L * 2 * 2 * 128], bf16, name="we2_s")
            nc.sync.dma_start(we2_s[:], we2_d[:])
            we2dr_s = csp.tile([128, L * 2 * 2 * 128], fp8, name="we2dr_s")
            nc.sync.dma_start(we2dr_s[:], we2dr_d[:])
            w1ab_s = csp.tile([128, L * 2 * 2 * 256], bf16, name="w1ab_s")
            nc.sync.dma_start(w1ab_s[:], w1ab_d[:])
            w1dp_s = csp.tile([128, L * 256], bf16, name="w1dp_s")
            nc.sync.dma_start(w1dp_s[64:128, :], w1dpv3_d[:])
            wn1a_s = csp.tile([128, L * 2 * 2 * 128], bf16, name="wn1a_s")
            nc.sync.dma_start(wn1a_s[:], wn1a_d[:])
            wn1b_s = csp.tile([128, L * 2 * 2 * 128], bf16, name="wn1b_s")
            nc.sync.dma_start(wn1b_s[:], wn1b_d[:])
            wn2_s = csp.tile([128, L * 2 * 2 * 128], bf16, name="wn2_s")
            nc.sync.dma_start(wn2_s[:], wn2_d[:])
            bias_s = csp.tile([128, 5 * L * 2], f32, name="bias_s")
            nc.sync.dma_start(bias_s[:], biasv3_d[:])
            zcol_s = csp.tile([128, 1], f32, name="zcol_s")
            nc.gpsimd.memset(zcol_s[:], 0.0)

            def w1ab_ap(l, sel, kc):
                o = ((l * 2 + sel) * 2 + kc) * 256
                return w1ab_s[:, o:o + 256]

            def we2dr_ap(l, mc):
                o = (l * 2 + mc) * 256
                return we2dr_s[:, o:o + 256].rearrange("p (k m) -> p k m", k=2)

            def we2_ap(l, kc, mc):
                o = ((l * 2 + kc) * 2 + mc) * 128
                return we2_s[:, o:o + 128]

            def wfam_ap(t, l, kc, mc):
                o = ((l * 2 + kc) * 2 + mc) * 128
                return t[:, o:o + 128]

            def bv3(j, l, mc):
                o = (j * L + l) * 2 + mc
                return bias_s[:, o:o + 1]

            hT, hbf, aggbf, n1bf = [], [], [], []
            for c in range(2):
                hT.append(stp.tile([128, nn], f32, name=f"hT{c}", tag=f"hT{c}"))
                hbf.append(stp.tile([128, nn], bf16, name=f"hbf{c}", tag=f"hbf{c}"))
                aggbf.append(stp.tile([128, nn], bf16, name=f"agb{c}", tag=f"agb{c}"))
                n1bf.append(stp.tile([128, nn], bf16, name=f"n1b{c}", tag=f"n1b{c}"))

            if not hw_loop:
                for c in range(2):
                    for t in range(nt):
                        sl = slice(t * nts, (t + 1) * nts)
                        nc.sync.dma_start(hT[c][:, sl], h0T_d[c][:, sl])
                        nc.gpsimd.tensor_copy(hbf[c][:, sl], hT[c][:, sl])

            crg_s = []
            for b in range(nb):
                t = crgp.tile([128, EPB], bf16, name=f"crg{b}", tag=f"crg{b}")
                nc.sync.dma_start(t[:], crg_d[b])
                crg_s.append(t)

            comb_tiles = [
                [combp.tile([128, 256], bf16, tag=f"comb{l}_{i}",
                            name=f"comb{l}_{i}") for i in range(min(4, nb))]
                for l in range(L)]

            # engine dispatch: 'A' = ACT, 'D' = DVE, 'P' = Pool/gpsimd
            ev_i = {"m1": 0, "m2": 0, "comb": 0, "red": 0}
            pats = {"m1": m1_pat, "m2": m2_pat, "comb": comb_pat, "red": red_pat}

            def eng(kind):
                ch = pats[kind][ev_i[kind] % len(pats[kind])]
                ev_i[kind] += 1
                return ch

            def evict_comb(out_ap, ps_ap, scale):
                ch = eng("comb")
                if ch == "A":
                    nc.scalar.mul(out_ap, ps_ap, scale)
                elif ch == "D":
                    nc.vector.tensor_scalar_mul(out_ap, ps_ap, scale)
                else:
                    nc.gpsimd.tensor_scalar_mul(out_ap, ps_ap, scale)

            def evict_m1(out_ap, ps_ap, l, kc):
                ch = eng("m1")
                if ch == "A":
                    nc.scalar.activation(out_ap, ps_ap, AF.Relu,
                                         bias=bv3(0, l, kc))
                else:
                    e = nc.vector if ch == "D" else nc.gpsimd
                    e.scalar_tensor_tensor(
                        out_ap, ps_ap, bv3(1, l, kc),
                        bv3(0, l, kc).to_broadcast(
                            [out_ap.shape[0], out_ap.free_size()]),
                        op0=ALU.max, op1=ALU.add)

            def evict_m2(out_ap, ps_ap, l, mc, s2):
                ch = "A" if be2_nz else eng("m2")
                if ch == "A":
                    nc.scalar.activation(out_ap, ps_ap, AF.Relu,
                                         bias=bv3(2, l, mc), scale=s2)
                else:
                    e = nc.vector if ch == "D" else nc.gpsimd
                    e.tensor_scalar(out_ap, ps_ap, s2, 0.0,
                                    op0=ALU.mult, op1=ALU.max)

            import contextlib as _ctx
            loop_ctx = (tc.For_i(0, reps, 1) if hw_loop else _ctx.nullcontext())
            rep_range = range(1 if hw_loop else reps)
            with loop_ctx:
             for rep in rep_range:
              if hw_loop or rep > 0:
                for c in range(2):
                    for t in range(nt):
                        sl = slice(t * nts, (t + 1) * nts)
                        nc.sync.dma_start(hT[c][:, sl], h0T_d[c][:, sl])
                        nc.gpsimd.tensor_copy(hbf[c][:, sl], hT[c][:, sl])
              for l in range(L):
                is_dr = l in dr_layers
                s2 = 1.0 / (_cm(l, dr_layers) * CW) if is_dr else 1.0
                for i in range(min(4, nb)):
                    nc.gpsimd.tensor_copy(
                        comb_tiles[l][i][64:128, :],
                        w1dp_s[64:128, l * 256:(l + 1) * 256])

                def emit_A(b):
                    ps_ab = psp.tile([128, 512], f32, tag="psA", bufs=2,
                                     name=f"ab{l}_{b}")
                    for sel, pos0 in ((0, 0), (1, 32)):
                        for kc in range(2):
                            nc.tensor.matmul(
                                ps_ab[pos0:pos0 + 32, 0:256],
                                lhsT=hbf[kc][:, b * NPB:(b + 1) * NPB],
                                rhs=w1ab_ap(l, sel, kc),
                                start=(kc == 0), stop=(kc == 1),
                                tile_position=(0, pos0),
                                skip_group_check=True)
                    comb = comb_tiles[l][b % 4]
                    evict_comb(comb[0:64, :], ps_ab[0:64, 0:256],
                               _cm(l, dr_layers))

                def emit_B(b):
                    if is_dr:
                        m1t = m1p.tile([128, 2, EPB], fp8, tag="m1f",
                                       name=f"m1_{l}_{b}")
                        m1aps = [m1t[:, kc, :] for kc in range(2)]
                    else:
                        m1a = m1p.tile([128, EPB], bf16, tag="m1a",
                                       name=f"m1a_{l}_{b}")
                        m1b = m1p.tile([128, EPB], bf16, tag="m1b",
                                       name=f"m1b_{l}_{b}")
                        m1t = [m1a, m1b]
                        m1aps = m1t
                    comb = comb_tiles[l][b % 4]
                    for kc in range(2):
                        for h in range(2):
                            ps1 = psp.tile([128, 512], f32, tag="ps1", bufs=2,
                                           name=f"p1_{l}_{b}_{kc}_{h}")
                            nc.tensor.matmul(
                                ps1[:, 0:HALF],
                                lhsT=comb[:, kc * 128:(kc + 1) * 128],
                                rhs=crg_s[b][:, h * HALF:(h + 1) * HALF],
                                start=True, stop=True)
                            evict_m1(m1aps[kc][:, h * HALF:(h + 1) * HALF],
                                     ps1[:, 0:HALF], l, kc)
                    return m1t

                def emit_C(b, m1t):
                    for mc in range(2):
                        for h in range(2):
                            hs = slice(h * HALF, (h + 1) * HALF)
                            ps2 = psp.tile([128, 512], f32, tag="psm2", bufs=4,
                                           name=f"p2_{l}_{b}_{mc}_{h}")
                            if is_dr:
                                nc.tensor.matmul(
                                    ps2[:, 0:HALF], lhsT=we2dr_ap(l, mc),
                                    rhs=m1t[:, :, hs],
                                    start=True, stop=True, perf_mode=DR)
                            else:
                                for kc in range(2):
                                    nc.tensor.matmul(
                                        ps2[:, 0:HALF],
                                        lhsT=we2_ap(l, kc, mc),
                                        rhs=m1t[kc][:, hs],
                                        start=(kc == 0), stop=(kc == 1))
                            m2sb = m2p.tile([128, HALF], bf16, tag="m2",
                                            name=f"m2_{l}_{b}_{mc}_{h}")
                            evict_m2(m2sb[:], ps2[:, 0:HALF], l, mc, s2)
                            with nc.allow_low_precision("bf16 agg as v2"):
                                nc.vector.tensor_reduce(
                                    aggbf[mc][:, b * NPB + h * 16:
                                              b * NPB + (h + 1) * 16],
                                    m2sb[:].rearrange("p (n k) -> p n k", k=K),
                                    axis=mybir.AxisListType.X, op=ALU.add)

                # software pipeline: A leads by 1 block, C lags by 1 block
                emit_A(0)
                m1_prev = None
                for b in range(nb):
                    m1_cur = emit_B(b)
                    if b + 1 < nb:
                        emit_A(b + 1)
                    if m1_prev is not None:
                        emit_C(b - 1, m1_prev)
                    m1_prev = m1_cur
                emit_C(nb - 1, m1_prev)

                # ---------------- node phase
                for mc in range(2):
                    for t in range(nt):
                        sl = slice(t * nts, (t + 1) * nts)
                        psn = psp.tile([128, 512], f32, tag="ps1", bufs=2,
                                       name=f"n1_{l}_{mc}_{t}")
                        for kc in range(2):
                            nc.tensor.matmul(psn[:, 0:nts],
                                             lhsT=wfam_ap(wn1a_s, l, kc, mc),
                                             rhs=hbf[kc][:, sl],
                                             start=(kc == 0), stop=False)
                            nc.tensor.matmul(psn[:, 0:nts],
                                             lhsT=wfam_ap(wn1b_s, l, kc, mc),
                                             rhs=aggbf[kc][:, sl],
                                             start=False, stop=(kc == 1))
                        nc.scalar.activation(n1bf[mc][:, sl], psn[:, 0:nts],
                                             AF.Relu, bias=bv3(4, l, mc))
                for mc in range(2):
                    for t in range(nt):
                        sl = slice(t * nts, (t + 1) * nts)
                        pso = psp.tile([128, 512], f32, tag="psm2", bufs=4,
                                       name=f"n2_{l}_{mc}_{t}")
                        for kc in range(2):
                            nc.tensor.matmul(pso[:, 0:nts],
                                             lhsT=wfam_ap(wn2_s, l, kc, mc),
                                             rhs=n1bf[kc][:, sl],
                                             start=(kc == 0), stop=(kc == 1))
                        nc.vector.scalar_tensor_tensor(
                            hT[mc][:, sl], hT[mc][:, sl], 2.0, pso[:, 0:nts],
                            op0=ALU.mult, op1=ALU.add)
                        if l + 1 < L:
                            nc.gpsimd.tensor_copy(hbf[mc][:, sl], hT[mc][:, sl])
              # ---------------- pooling
              for mc in range(2):
                pool_t = stp.tile([128, nb], f32, tag=f"pool{mc}", name=f"pool{mc}")
                nc.vector.tensor_reduce(pool_t[:],
                                        hT[mc][:].rearrange("p (n k) -> p n k", k=NPB),
                                        axis=mybir.AxisListType.X, op=ALU.add)
                nc.scalar.mul(pool_t[:], pool_t[:], 1.0 / NPB)
                nc.sync.dma_start(out_d[mc], pool_t[:])
    return nc


# --------------------------------------------------------------- builder v4
# v2 pass structure with: bias matmuls removed (bn1 via ACT bias; bn2 must
# be zero), pass-D ssel matmuls interleaved one-for-one behind the next
# group's pass-C matmuls (hides D's LDWEIGHTS exposure), persistent
# ping-pong agg PSUM halves (no group serialization), pass A packing two
# blocks per PSUM tile via 4 col-strips, PSUM evictions on ACT/DVE only.

def build_nc_v4(nb=BPC, reps=1, hw_loop=False, be1_nz=False,
                comb_pat="D", m1_pat="ADAD", m2_pat="AADADAADDA", agg_pat="D",
                drain_c=1, drain_n=3):
    import contextlib
    import concourse.bass as bass
    import concourse.mybir as mybir
    import concourse.tile as tile

    f32, bf16 = mybir.dt.float32, mybir.dt.bfloat16
    AF = mybir.ActivationFunctionType
    ALU = mybir.AluOpType
    nn = nb * NPB
    nts = min(512, nn)
    nt = nn // nts
    ngr = max(1, nb // 4)
    assert nb % 4 == 0 or nb == 2

    nc = bass.Bass()
    h0T_d = nc.dram_tensor("h0T", [2, 128, nn], f32, kind="ExternalInput")
    crg_d = nc.dram_tensor("crg", [nb, 128, EPB], bf16, kind="ExternalInput")
    we2r_d = nc.dram_tensor("we2r", [128, L * 2 * 256], bf16, kind="ExternalInput")
    w1ab_d = nc.dram_tensor("w1ab", [128, L * 2 * 2 * 256], bf16,
                            kind="ExternalInput")
    w1dp_d = nc.dram_tensor("w1dp", [64, L * 256], bf16, kind="ExternalInput")
    wn1a_d = nc.dram_tensor("wn1a", [128, L * 2 * 2 * 128], bf16,
                            kind="ExternalInput")
    wn1b_d = nc.dram_tensor("wn1b", [128, L * 2 * 2 * 128], bf16,
                            kind="ExternalInput")
    wn2_d = nc.dram_tensor("wn2", [128, L * 2 * 2 * 128], bf16,
                           kind="ExternalInput")
    biasv3_d = nc.dram_tensor("biasv3", [128, 5 * L * 2], f32,
                              kind="ExternalInput")
    ssel_d = nc.dram_tensor("ssel", [128, 5 * NPB], bf16, kind="ExternalInput")
    out_d = nc.dram_tensor("poolT", [2, 128, nb], f32, kind="ExternalOutput")

    with tile.TileContext(nc) as tc:
        with (
            tc.tile_pool(name="const", bufs=1) as csp,
            tc.tile_pool(name="crgp", bufs=1) as crgp,
            tc.tile_pool(name="state", bufs=1) as stp,
            tc.tile_pool(name="comb", bufs=1) as combp,
            tc.tile_pool(name="m1p", bufs=5) as m1p,
            tc.tile_pool(name="m2p", bufs=26) as m2p,
            tc.tile_pool(name="ps", bufs=7, space="PSUM") as psp,
            tc.tile_pool(name="psagg", bufs=1, space="PSUM") as psaggp,
        ):
            we2r_s = csp.tile([128, L * 2 * 256], bf16, name="we2r_s")
            nc.sync.dma_start(we2r_s[:], we2r_d[:])
            w1ab_s = csp.tile([128, L * 2 * 2 * 256], bf16, name="w1ab_s")
            nc.sync.dma_start(w1ab_s[:], w1ab_d[:])
            w1dp_s = csp.tile([128, L * 256], bf16, name="w1dp_s")
            nc.sync.dma_start(w1dp_s[64:128, :], w1dp_d[:])
            wn1a_s = csp.tile([128, L * 2 * 2 * 128], bf16, name="wn1a_s")
            nc.sync.dma_start(wn1a_s[:], wn1a_d[:])
            wn1b_s = csp.tile([128, L * 2 * 2 * 128], bf16, name="wn1b_s")
            nc.sync.dma_start(wn1b_s[:], wn1b_d[:])
            wn2_s = csp.tile([128, L * 2 * 2 * 128], bf16, name="wn2_s")
            nc.sync.dma_start(wn2_s[:], wn2_d[:])
            bias_s = csp.tile([128, 5 * L * 2], f32, name="bias_s")
            nc.sync.dma_start(bias_s[:], biasv3_d[:])
            ssel_s = csp.tile([128, 5 * NPB], bf16, name="ssel_s")
            nc.sync.dma_start(ssel_s[:], ssel_d[:])

            def we2r_ap(l, kc):
                o = (l * 2 + kc) * 256
                return we2r_s[:, o:o + 256]

            def w1ab_ap(l, sel, kc):
                o = ((l * 2 + sel) * 2 + kc) * 256
                return w1ab_s[:, o:o + 256]

            def wfam_ap(t, l, kc, mc):
                o = ((l * 2 + kc) * 2 + mc) * 128
                return t[:, o:o + 128]

            def bv3(j, l, mc):
                o = (j * L + l) * 2 + mc
                return bias_s[:, o:o + 1]

            hT, hbf, aggbf, n1bf = [], [], [], []
            for c in range(2):
                hT.append(stp.tile([128, nn], f32, name=f"hT{c}", tag=f"hT{c}"))
                hbf.append(stp.tile([128, nn], bf16, name=f"hbf{c}", tag=f"hbf{c}"))
                aggbf.append(stp.tile([128, nn], bf16, name=f"agb{c}", tag=f"agb{c}"))
                n1bf.append(stp.tile([128, nn], bf16, name=f"n1b{c}", tag=f"n1b{c}"))

            agg_ps = psaggp.tile([128, 512], f32, name="agg_ps", tag="aggps")

            if not hw_loop:
                for c in range(2):
                    for t in range(nt):
                        sl = slice(t * nts, (t + 1) * nts)
                        nc.sync.dma_start(hT[c][:, sl], h0T_d[c][:, sl])
                        nc.gpsimd.tensor_copy(hbf[c][:, sl], hT[c][:, sl])

            crg_s = []
            for b in range(nb):
                t = crgp.tile([128, EPB], bf16, name=f"crg{b}", tag=f"crg{b}")
                nc.sync.dma_start(t[:], crg_d[b])
                crg_s.append(t)

            comb_tiles = [
                [combp.tile([128, 256], bf16, tag=f"comb{l}_{i}",
                            name=f"comb{l}_{i}") for i in range(min(4, nb))]
                for l in range(L)]

            ev_i = {"comb": 0, "m1": 0, "m2": 0, "agg": 0}
            pats = {"comb": comb_pat, "m1": m1_pat, "m2": m2_pat, "agg": agg_pat}

            def eng(kind):
                ch = pats[kind][ev_i[kind] % len(pats[kind])]
                ev_i[kind] += 1
                return ch

            def ev_copy(kind, out_ap, ps_ap):
                if eng(kind) == "A":
                    nc.scalar.copy(out_ap, ps_ap)
                else:
                    nc.vector.tensor_copy(out_ap, ps_ap)

            def ev_relu(kind, out_ap, ps_ap, jpos, jneg, l, mc):
                if eng(kind) == "A":
                    nc.scalar.activation(out_ap, ps_ap, AF.Relu,
                                         bias=bv3(jpos, l, mc))
                else:
                    nc.vector.scalar_tensor_tensor(
                        out_ap, ps_ap, bv3(jneg, l, mc),
                        bv3(jpos, l, mc).to_broadcast(
                            [out_ap.shape[0], out_ap.free_size()]),
                        op0=ALU.max, op1=ALU.add)

            def ev_relu0(kind, out_ap, ps_ap):
                # bias-free relu (valid only when the bias is zero)
                if eng(kind) == "A":
                    nc.scalar.activation(out_ap, ps_ap, AF.Relu)
                else:
                    nc.vector.tensor_scalar(out_ap, ps_ap, 0.0, None,
                                            op0=ALU.max)

            # D-instruction queue (thunks); drained behind later PE work
            dq = []

            def drain(k):
                for _ in range(min(k, len(dq))):
                    dq.pop(0)()

            loop_ctx = (tc.For_i(0, reps, 1) if hw_loop else contextlib.nullcontext())
            rep_range = range(1 if hw_loop else reps)
            with loop_ctx:
             for rep in rep_range:
              if hw_loop or rep > 0:
                for c in range(2):
                    for t in range(nt):
                        sl = slice(t * nts, (t + 1) * nts)
                        nc.sync.dma_start(hT[c][:, sl], h0T_d[c][:, sl])
                        nc.gpsimd.tensor_copy(hbf[c][:, sl], hT[c][:, sl])
              for l in range(L):
                for i in range(min(4, nb)):
                    nc.gpsimd.tensor_copy(
                        comb_tiles[l][i][64:128, :],
                        w1dp_s[64:128, l * 256:(l + 1) * 256])

                def emit_A(g):
                    # two blocks per PSUM tile, 4 col-strips
                    for half in range(2):
                        b0 = g * 4 + half * 2
                        if b0 >= nb:
                            return
                        ps_ab = psp.tile([128, 512], f32, tag="ps",
                                         name=f"ab{l}_{b0}")
                        for bi in range(2):
                            b = b0 + bi
                            if b >= nb:
                                break
                            for sel in range(2):
                                pos0 = bi * 64 + sel * 32
                                for kc in range(2):
                                    nc.tensor.matmul(
                                        ps_ab[pos0:pos0 + 32, 0:256],
                                        lhsT=hbf[kc][:, b * NPB:(b + 1) * NPB],
                                        rhs=w1ab_ap(l, sel, kc),
                                        start=(kc == 0), stop=(kc == 1),
                                        tile_position=(0, pos0),
                                        skip_group_check=True)
                        for bi in range(2):
                            b = b0 + bi
                            if b >= nb:
                                break
                            comb = comb_tiles[l][b % 4]
                            ev_copy("comb", comb[0:64, :],
                                    ps_ab[bi * 64:bi * 64 + 64, 0:256])

                def emit_B(b):
                    comb = comb_tiles[l][b % 4]
                    m1t = m1p.tile([128, 2, EPB], bf16, tag="m1",
                                   name=f"m1_{l}_{b}")
                    if be1_nz:
                        # per-kc bias forces per-(kc,h) evictions
                        for kc in range(2):
                            for h in range(2):
                                ps1 = psp.tile([128, 512], f32, tag="ps",
                                               name=f"p1_{l}_{b}_{kc}_{h}")
                                nc.tensor.matmul(
                                    ps1[:, 0:HALF],
                                    lhsT=comb[:, kc * 128:(kc + 1) * 128],
                                    rhs=crg_s[b][:, h * HALF:(h + 1) * HALF],
                                    start=True, stop=True,
                                    skip_group_check=True)
                                ev_relu("m1",
                                        m1t[:, kc, h * HALF:(h + 1) * HALF],
                                        ps1[:, 0:HALF], 0, 1, l, kc)
                        return m1t
                    # be1 == 0: edges chunked 512+128 per kc; the two kc
                    # 128-tails share one PSUM tile -> 3 evictions, not 4
                    tails = psp.tile([128, 512], f32, tag="ps",
                                     name=f"p1t_{l}_{b}")
                    for kc in range(2):
                        ps1 = psp.tile([128, 512], f32, tag="ps",
                                       name=f"p1_{l}_{b}_{kc}")
                        nc.tensor.matmul(
                            ps1[:, 0:512],
                            lhsT=comb[:, kc * 128:(kc + 1) * 128],
                            rhs=crg_s[b][:, 0:512],
                            start=True, stop=True, skip_group_check=True)
                        nc.tensor.matmul(
                            tails[:, kc * 128:(kc + 1) * 128],
                            lhsT=comb[:, kc * 128:(kc + 1) * 128],
                            rhs=crg_s[b][:, 512:EPB],
                            start=True, stop=True, skip_group_check=True)
                        ev_relu("m1", m1t[:, kc, 0:512], ps1[:, 0:512],
                                0, 1, l, kc)
                    ev_relu0("m1", m1t[:, :, 512:EPB],
                             tails[:, 0:256].rearrange("p (k e) -> p k e", k=2))
                    return m1t

                def emit_C(b, m1t):
                    m2sbs = []
                    for p in range(3):
                        ecs = (2 * p, 2 * p + 1) if p < 2 else (4,)
                        w = 256 * len(ecs)
                        ps2 = psp.tile([128, 512], f32, tag="ps",
                                       name=f"p2_{l}_{b}_{p}")
                        for j, ec in enumerate(ecs):
                            for kc in range(2):
                                nc.tensor.matmul(
                                    ps2[:, j * 256:(j + 1) * 256],
                                    lhsT=m1t[:, kc,
                                             ec * 128:(ec + 1) * 128],
                                    rhs=we2r_ap(l, kc),
                                    start=(kc == 0), stop=(kc == 1),
                                    skip_group_check=True)
                                drain(drain_c)
                        m2sb = m2p.tile([128, 512], bf16, tag="m2",
                                        name=f"m2_{l}_{b}_{p}")
                        for j in range(len(ecs)):
                            jm = j * 256
                            ev_relu("m2", m2sb[:, jm:jm + 256],
                                    ps2[:, jm:jm + 256], 2, 3, l, 0)
                        m2sbs.append(m2sb)
                    return m2sbs

                def queue_D(g, m2ss):
                    half = (g % 2) * 256

                    def mk_mm(bi, mc, ec):
                        p, j = divmod(ec, 2)
                        m2sb = m2ss[bi]

                        def f():
                            nc.tensor.matmul(
                                agg_ps[:, half + mc * 128 + bi * 32:
                                       half + mc * 128 + bi * 32 + 32],
                                lhsT=m2sb[p][:, j * 256 + mc * 128:
                                             j * 256 + (mc + 1) * 128],
                                rhs=ssel_s[:, ec * NPB:(ec + 1) * NPB],
                                start=(ec == 0), stop=(ec == 4),
                                skip_group_check=True)
                        return f

                    for bi in range(min(4, nb)):
                        for mc in range(2):
                            for ec in range(5):
                                dq.append(mk_mm(bi, mc, ec))

                    def mk_ev(mc):
                        def f():
                            ev_copy("agg", aggbf[mc][:, g * 128:(g + 1) * 128],
                                    agg_ps[:, half + mc * 128:
                                           half + (mc + 1) * 128])
                        return f
                    for mc in range(2):
                        dq.append(mk_ev(mc))

                # ---------------- edge phase, software-pipelined groups
                emit_A(0)
                for g in range(ngr):
                    m1ts = {}
                    for bi in range(min(4, nb)):
                        m1ts[bi] = emit_B(g * 4 + bi)
                    if g + 1 < ngr:
                        emit_A(g + 1)
                    m2ss = {}
                    for bi in range(min(4, nb)):
                        m2ss[bi] = emit_C(g * 4 + bi, m1ts[bi])
                        drain(2)
                    queue_D(g, m2ss)
                # ---------------- node phase (drains the last D group)
                for t in range(nt):
                    if t == nt - 1:
                        drain(len(dq))
                    for mc in range(2):
                        sl = slice(t * nts, (t + 1) * nts)
                        psn = psp.tile([128, 512], f32, tag="ps",
                                       name=f"n1_{l}_{mc}_{t}")
                        for kc in range(2):
                            nc.tensor.matmul(psn[:, 0:nts],
                                             lhsT=wfam_ap(wn1a_s, l, kc, mc),
                                             rhs=hbf[kc][:, sl],
                                             start=(kc == 0), stop=False,
                                             skip_group_check=True)
                            drain(drain_n)
                            nc.tensor.matmul(psn[:, 0:nts],
                                             lhsT=wfam_ap(wn1b_s, l, kc, mc),
                                             rhs=aggbf[kc][:, sl],
                                             start=False, stop=(kc == 1),
                                             skip_group_check=True)
                            drain(drain_n)
                        nc.scalar.activation(n1bf[mc][:, sl], psn[:, 0:nts],
                                             AF.Relu, bias=bv3(4, l, mc))
                drain(len(dq))
                for t in range(nt):
                    for mc in range(2):
                        sl = slice(t * nts, (t + 1) * nts)
                        pso = psp.tile([128, 512], f32, tag="ps",
                                       name=f"n2_{l}_{mc}_{t}")
                        for kc in range(2):
                            nc.tensor.matmul(pso[:, 0:nts],
                                             lhsT=wfam_ap(wn2_s, l, kc, mc),
                                             rhs=n1bf[kc][:, sl],
                                             start=(kc == 0), stop=(kc == 1),
                                             skip_group_check=True)
                        nc.vector.scalar_tensor_tensor(
                            hT[mc][:, sl], hT[mc][:, sl], 2.0, pso[:, 0:nts],
                            op0=ALU.mult, op1=ALU.add)
                        if l + 1 < L:
                            nc.gpsimd.tensor_copy(hbf[mc][:, sl], hT[mc][:, sl])
              # ---------------- pooling
              for mc in range(2):
                pool_t = stp.tile([128, nb], f32, tag=f"pool{mc}", name=f"pool{mc}")
                nc.vector.tensor_reduce(pool_t[:],
                                        hT[mc][:].rearrange("p (n k) -> p n k", k=NPB),
                                        axis=mybir.AxisListType.X, op=ALU.add)
                nc.scalar.mul(pool_t[:], pool_t[:], 1.0 / NPB)
                nc.sync.dma_start(out_d[mc], pool_t[:])
    return nc


# ---------------------------------------------- numpy model of the v3 math

def numpy_model_v3(ins, extras, nb=BPC, cores=None, dr_layers=DR_LAYERS):
    E4 = ml_dtypes.float8_e4m3fn

    def b16(x):
        return x.astype(BF16).astype(np.float32)

    def f8(x):
        return x.astype(E4).astype(np.float32)

    we2dr = np.asarray(extras["we2dr"], np.float32).reshape(128, L, 2, 2, 128)
    w1dpv3 = np.asarray(extras["w1dpv3"], np.float32).reshape(64, L, 256)
    bias = np.asarray(extras["biasv3"], np.float32)

    def bcol(j, l):
        return np.concatenate([bias[:, (j * L + l) * 2 + mc] for mc in range(2)])

    outs = []
    for m in (ins if cores is None else [ins[c] for c in cores]):
        h = np.asarray(m["h0T"], np.float32).reshape(256, -1)[:, :nb * NPB]
        crg = np.asarray(m["crg"], np.float32)[:nb]
        w1ab = np.asarray(m["w1ab"], np.float32).reshape(
            128, L, 2, 2, 256).transpose(1, 2, 3, 0, 4)
        we2 = np.asarray(m["we2"], np.float32).reshape(
            128, L, 2, 2, 128).transpose(1, 2, 3, 0, 4)
        wn1a = np.asarray(m["wn1a"], np.float32).reshape(
            128, L, 2, 2, 128).transpose(1, 2, 3, 0, 4)
        wn1b = np.asarray(m["wn1b"], np.float32).reshape(
            128, L, 2, 2, 128).transpose(1, 2, 3, 0, 4)
        wn2 = np.asarray(m["wn2"], np.float32).reshape(
            128, L, 2, 2, 128).transpose(1, 2, 3, 0, 4)
        nn = nb * NPB

        def blk(w):
            return np.concatenate(
                [np.concatenate([w[kc_, mc_] for mc_ in range(2)], axis=1)
                 for kc_ in range(2)], axis=0)

        for l in range(L):
            is_dr = l in dr_layers
            cm = _cm(l, dr_layers)
            hb = b16(h)
            W1b = np.concatenate([w1ab[l, 0, kc_] for kc_ in range(2)], axis=0)
            W1a = np.concatenate([w1ab[l, 1, kc_] for kc_ in range(2)], axis=0)
            if is_dr:
                # We2 pack slice (l, mc): [128, kc, m]; logical [256, 256]
                W2 = np.concatenate(
                    [np.concatenate([we2dr[:, l, mc, kc, :] for mc in range(2)],
                                    axis=1) for kc in range(2)], axis=0)
                s2 = 1.0 / (cm * CW)
            else:
                W2 = blk(we2[l])
                s2 = 1.0
            be1c = bcol(0, l)
            be2 = bcol(2, l)
            bn1 = bcol(4, l)
            agg = np.zeros((256, nn), np.float32)
            for b in range(nb):
                hs = hb[:, b * NPB:(b + 1) * NPB]
                hB = b16(hs.T @ W1b * cm)
                hA = b16(hs.T @ W1a * cm)
                combined = np.concatenate([hB, hA, w1dpv3[:, l, :]], axis=0)
                pre1 = combined.T @ crg[b]
                m1 = np.maximum(pre1 + be1c[:, None], 0.0)
                m1 = f8(m1) if is_dr else b16(m1)
                m2 = b16(np.maximum(W2.T @ m1 * s2 + be2[:, None], 0.0))
                agg[:, b * NPB:(b + 1) * NPB] = b16(
                    m2.reshape(256, NPB, K).sum(axis=2))
            aggb = agg
            N1a, N1b_, N2 = blk(wn1a[l]), blk(wn1b[l]), blk(wn2[l])
            n1 = b16(np.maximum(N1a.T @ hb + N1b_.T @ aggb + bn1[:, None], 0.0))
            out = N2.T @ n1
            h = 2.0 * h + out
        pooled = h.reshape(256, nb, NPB).mean(axis=2)
        outs.append(pooled.T)
    return np.concatenate(outs, axis=0)


# ===================================================================== entry

_CACHE = {}


def _get_runner(be2_mm):
    key = ("runner", be2_mm)
    if key not in _CACHE:
        apply_tilefix()
        nc = build_nc_v2(nb=BPC, be2_mm=be2_mm,
                         m1_dve_of_8=4, m2_dve_of_8=4,
                         comb_dve_of_8=5, agg_dve_of_8=2)
        split_waits(nc, cap=1, cap_sp=1)
        _CACHE[key] = nc
    return _CACHE[key]


def _get_runner_v4(be1_nz=False):
    key = ("runner_v4", be1_nz)
    if key not in _CACHE:
        apply_tilefix()
        nc = build_nc_v4(nb=BPC, be1_nz=be1_nz)
        split_waits(nc, cap=1, cap_sp=1)
        _CACHE[key] = nc
    return _CACHE[key]


def _run(nc, per_core):
    import concourse.mybir as mybir
    from concourse.bass_utils import run_bass_kernel_spmd
    declared = set()
    for alloc in nc.m.functions[0].allocations:
        if isinstance(alloc, mybir.MemoryLocationSet) and alloc.kind == "ExternalInput":
            declared.add(alloc.memorylocations[0].name)
    in_maps = [{k: v for k, v in m.items() if k in declared} for m in per_core]
    res = run_bass_kernel_spmd(nc, in_maps, core_ids=list(range(NCORES)))
    return host_unshard(res.results).astype(np.float32)


def kernel(**inputs):
    """Full inputs in (as in reference.setup_inputs), full [B, 256] f32 out."""
    np_inputs = {k: np.asarray(v) for k, v in inputs.items()}
    per_core = host_prep(**np_inputs)
    if np.any(np.asarray(np_inputs["bn2"]) != 0):
        # v4 folds bn2 away only when it is zero; exact fallback to v2
        be2_mm = bool(per_core[0]["be2_nonzero"][0])
        return _run(_get_runner(be2_mm), per_core)
    extras = host_prep_v3(np_inputs)
    per_core = [{**m, **extras} for m in per_core]
    be1_nz = bool(np.any(np.asarray(np_inputs["be1"]) != 0))
    return _run(_get_runner_v4(be1_nz), per_core)

